# revision 1
# baseline (speedup 1.0000x reference)
"""Causal single-head attention on 8 TRN2 NeuronCores.

Problem: x [4, 2048, 768] f32; Wq/Wk/Wv [768, 768] f32 (torch Linear layout).
  q/k/v = x @ W.T ; scores = q k^T causal-masked; attn = softmax(scores/sqrt(768));
  out = attn @ v.

Sharding: core c -> batch b = c//2, half h = c%2. The two cores of a batch
split the 16 query tiles (128 rows each) INTERLEAVED: core h owns global
q-tiles {2*lt + h : lt in 0..7}. Causal attention for global q-tile g only
needs keys 0 .. 128*(g+1), i.e. ceil((g+1)/4) 512-wide key chunks; with the
even/odd interleave both cores see the identical chunk-count sequence
[1,1,2,2,3,3,4,4], so the SPMD program is uniform across cores while doing
EXACT causal work (no fully-masked chunks are ever computed). Only the
diagonal chunk of each q-tile needs masking; its 4 possible within-chunk
patterns are passed as a small per-core strip input ([128, 1024], window
picked by lt%2).

The host passes x^T (global key order, shared by the pair), xq^T (the core's
own interleaved query rows), and W^T - host transposes are pure layout prep.
Matmuls run in float32r (TensorE fast-fp32, 4x the fp32 rate at free dim
>= 256, ~2e-4 input rounding); raw fp32 bits feed float32r-typed DRAM inputs
directly - the PE converts on load, so the device does no transposes and no
rounding work at all.

Device pipeline per core:
  1. Q^T [768,1024] projected first (resident in SBUF), then stream x^T in
     512-col chunks -> K^T [768,2048] and V [2048,768] resident in SBUF;
     attention tiles can begin as soon as the first K/V chunks land.
  2. Per local q-tile lt (Nc = [1,1,2,2,3,3,4,4][lt] key chunks): scores via
     f32r matmuls; non-diagonal chunks exp directly from PSUM on ScalarE;
     the diagonal chunk gets a VectorE strip-add (fused evacuation) then exp;
     every exp emits its row-sum via accum_out (no max-subtraction: scaled
     scores are O(+-5), safely inside fp32 exp range). attn tiles transposed
     on TensorE 4-per-PSUM-bank; context accumulates over 4*Nc key tiles;
     softmax 1/rowsum is fused into the context PSUM evacuation.
"""

import os
import sys
from contextlib import ExitStack

import numpy as np

for _p in ("/opt/trn_rl_repo", "/root/.axon_site/_ro/trn_rl_repo"):
    if os.path.isdir(_p) and _p not in sys.path:
        sys.path.append(_p)

import concourse.mybir as mybir  # noqa: E402
import concourse.tile as tile  # noqa: E402
from concourse import bacc  # noqa: E402
from concourse.bass_utils import run_bass_kernel_spmd  # noqa: E402
from concourse.masks import make_identity  # noqa: E402

F32 = mybir.dt.float32
F32R = mybir.dt.float32r

BATCH = 4
SEQ = 2048
D = 768
DK = D // 128  # contraction chunks (6)
NQ = 1024  # query rows per core
LT = NQ // 128  # local q-tiles per core (8)
XC = 512  # streaming chunk width
NCS = [1, 1, 2, 2, 3, 3, 4, 4]  # key chunks per local q-tile (both cores!)
SCALE = 1.0 / float(np.sqrt(np.float32(D)))
NEG = -1e30

_CACHE = {}


def _build(repeat=1):
    nc = bacc.Bacc("TRN2", target_bir_lowering=False, debug=False, num_devices=8)
    xt_d = nc.declare_dram_parameter("xt", [D, SEQ], F32R, isOutput=False)
    xqt_d = nc.declare_dram_parameter("xqt", [D, NQ], F32R, isOutput=False)
    wqt_d = nc.declare_dram_parameter("wqt", [D, D], F32R, isOutput=False)
    wkt_d = nc.declare_dram_parameter("wkt", [D, D], F32R, isOutput=False)
    wvt_d = nc.declare_dram_parameter("wvt", [D, D], F32R, isOutput=False)
    strip_d = nc.declare_dram_parameter("strip", [128, 1024], F32, isOutput=False)
    out_d = nc.declare_dram_parameter("out", [NQ, D], F32, isOutput=True)

    # Rotate input DMAs across engines' DGE queues - a single queue serializes
    # the ~16MB of input transfers and stalls the PE at kernel start.
    _dma_i = [0]

    def dma_in(dst, src):
        eng = (nc.sync, nc.scalar)[_dma_i[0] % 2]
        eng.dma_start(dst, src)
        _dma_i[0] += 1

    # Round-robin PSUM evacuation between VectorE and ScalarE.
    _evac_i = [0]

    def evac(dst, src):
        if _evac_i[0] % 2 == 0:
            nc.vector.tensor_copy(dst, src)
        else:
            nc.scalar.copy(dst, src)
        _evac_i[0] += 1

    with tile.TileContext(nc) as tc, ExitStack() as ctx:
        persist = ctx.enter_context(tc.tile_pool(name="persist", bufs=1))

        ident = persist.tile([128, 128], F32)
        make_identity(nc, ident[:])

        strip = persist.tile([128, 1024], F32)
        nc.gpsimd.dma_start(strip[:], strip_d[:])

        kt = persist.tile([128, DK, SEQ], F32R)  # K^T
        vt = persist.tile([128, SEQ // 128, D], F32R)  # V (natural layout)
        qt_sb = persist.tile([128, DK, NQ], F32R)  # Q^T (resident)

        for _rep in range(repeat):
          # wk prefetches in a pool coexisting with the whole Q phase, so its
          # DMA is not blocked on the Q-phase SBUF region being released.
          with ExitStack() as p1:
            wkpool = p1.enter_context(tc.tile_pool(name="wkpool", bufs=1))
            wtk = wkpool.tile([128, DK, D], F32R, name="wtk")

            # ---------------- Phase 1b: Q^T projection (resident) ----------------
            with ExitStack() as p2b:
                wqpool = p2b.enter_context(tc.tile_pool(name="wqpool", bufs=1))
                xqc_p = p2b.enter_context(tc.tile_pool(name="xqc", bufs=2))
                ps_q = p2b.enter_context(
                    tc.tile_pool(name="ps_q", bufs=3, space="PSUM")
                )
                # wtq/xqc split into ko-halves as SEPARATE tiles: dependency
                # tracking is per-tile, so ko 0..2 matmuls start after half the
                # input bytes instead of waiting for the full load.
                wtqh = []
                for half in range(2):
                    wq_h = wqpool.tile([128, 3, D], F32R, name=f"wtq{half}")
                    dma_in(
                        wq_h[:],
                        wqt_d[half * 384 : (half + 1) * 384, :].rearrange(
                            "(ko p) o -> p ko o", p=128
                        ),
                    )
                    wtqh.append(wq_h)
                for sc in range(NQ // XC):
                    xqch = []
                    for half in range(2):
                        xq_h = xqc_p.tile([128, 3, XC], F32R, tag=f"xqc{half}")
                        dma_in(
                            xq_h[:],
                            xqt_d[
                                half * 384 : (half + 1) * 384,
                                sc * XC : (sc + 1) * XC,
                            ].rearrange("(ko p) s -> p ko s", p=128),
                        )
                        xqch.append(xq_h)
                    if sc == 0:
                        # prefetch W_k during the Q phase
                        for half in range(2):
                            dma_in(
                                wtk[:, half * 3 : (half + 1) * 3, :],
                                wkt_d[half * 384 : (half + 1) * 384, :].rearrange(
                                    "(ko p) o -> p ko o", p=128
                                ),
                            )
                    for oo in range(DK):
                        pq = ps_q.tile([128, XC], F32, tag="ps_q")
                        for ko in range(DK):
                            nc.tensor.matmul(
                                pq[:],
                                wtqh[ko // 3][:, ko % 3, oo * 128 : (oo + 1) * 128],
                                xqch[ko // 3][:, ko % 3, :],
                                start=(ko == 0),
                                stop=(ko == DK - 1),
                            )
                        nc.vector.tensor_copy(qt_sb[:, oo, sc * XC : (sc + 1) * XC], pq[:])

            # ---------------- Phase 1a: K^T / V projections ----------------
            with ExitStack() as p2:
                wvpool = p2.enter_context(tc.tile_pool(name="wvpool", bufs=1))
                xtc_p = p2.enter_context(tc.tile_pool(name="xtc", bufs=2))
                ps_p512 = p2.enter_context(
                    tc.tile_pool(name="ps_p512", bufs=3, space="PSUM")
                )
                ps_p384 = p2.enter_context(
                    tc.tile_pool(name="ps_p384", bufs=4, space="PSUM")
                )

                wtv = wvpool.tile([128, DK, D], F32R, name="wtv")

                for sc in range(SEQ // XC):
                    xtc = xtc_p.tile([128, DK, XC], F32R, tag="xtc")
                    for half in range(2):
                        dma_in(
                            xtc[:, half * 3 : (half + 1) * 3, :],
                            xt_d[
                                half * 384 : (half + 1) * 384,
                                sc * XC : (sc + 1) * XC,
                            ].rearrange("(ko p) s -> p ko s", p=128),
                        )
                    if sc == 0:
                        # W_v load queues behind xtc0 so K-chunk0 starts sooner
                        for half in range(2):
                            dma_in(
                                wtv[:, half * 3 : (half + 1) * 3, :],
                                wvt_d[half * 384 : (half + 1) * 384, :].rearrange(
                                    "(ko p) o -> p ko o", p=128
                                ),
                            )

                    # K^T chunk
                    for oo in range(DK):
                        pk = ps_p512.tile([128, XC], F32, tag="p512")
                        for ko in range(DK):
                            nc.tensor.matmul(
                                pk[:],
                                wtk[:, ko, oo * 128 : (oo + 1) * 128],
                                xtc[:, ko, :],
                                start=(ko == 0),
                                stop=(ko == DK - 1),
                            )
                        evac(kt[:, oo, sc * XC : (sc + 1) * XC], pk[:])

                    # V chunk: per 128-row seq tile, dout in two 384 halves
                    for st in range(XC // 128):
                        seq_tile = sc * (XC // 128) + st
                        for oc in range(2):
                            pv = ps_p384.tile([128, 384], F32, tag="p384")
                            for ko in range(DK):
                                nc.tensor.matmul(
                                    pv[:],
                                    xtc[:, ko, st * 128 : (st + 1) * 128],
                                    wtv[:, ko, oc * 384 : (oc + 1) * 384],
                                    start=(ko == 0),
                                    stop=(ko == DK - 1),
                                )
                            evac(vt[:, seq_tile, oc * 384 : (oc + 1) * 384], pv[:])

            # ---------------- Phase 2: attention per local q-tile ----------------
            with ExitStack() as p3:
                scd_p = p3.enter_context(tc.tile_pool(name="scd", bufs=3))
                attn_p = p3.enter_context(tc.tile_pool(name="attn", bufs=3))
                attnT_p = p3.enter_context(tc.tile_pool(name="attnT", bufs=3))
                ctx_p = p3.enter_context(tc.tile_pool(name="ctxs", bufs=3))
                small_p = p3.enter_context(tc.tile_pool(name="small", bufs=2))
                ps_s = p3.enter_context(tc.tile_pool(name="ps_s", bufs=3, space="PSUM"))
                ps_t3 = p3.enter_context(
                    tc.tile_pool(name="ps_t3", bufs=2, space="PSUM")
                )
                ps_c1 = p3.enter_context(
                    tc.tile_pool(name="ps_c1", bufs=2, space="PSUM")
                )
                ps_c2 = p3.enter_context(
                    tc.tile_pool(name="ps_c2", bufs=1, space="PSUM")
                )

                for lt in range(LT):
                    ncs = NCS[lt]
                    attn = attn_p.tile([128, SEQ], F32, tag="attn")
                    rs = small_p.tile([128, 4], F32, tag="rs")

                    for kc in range(ncs):
                        pss = ps_s.tile([128, 512], F32, tag="ps_s")
                        for ko in range(DK):
                            nc.tensor.matmul(
                                pss[:],
                                qt_sb[:, ko, lt * 128 : (lt + 1) * 128],
                                kt[:, ko, kc * 512 : (kc + 1) * 512],
                                start=(ko == 0),
                                stop=(ko == DK - 1),
                            )
                        if kc == ncs - 1:
                            # diagonal chunk: strip-add (VectorE, fused evac), then exp
                            scd = scd_p.tile([128, 512], F32, tag="scd")
                            nc.vector.tensor_add(
                                scd[:],
                                pss[:],
                                strip[:, (lt % 2) * 512 : (lt % 2) * 512 + 512],
                            )
                            nc.scalar.activation(
                                attn[:, kc * 512 : (kc + 1) * 512],
                                scd[:],
                                mybir.ActivationFunctionType.Exp,
                                scale=SCALE,
                                accum_out=rs[:, kc : kc + 1],
                            )
                        else:
                            # interior chunk: exp straight from PSUM
                            nc.scalar.activation(
                                attn[:, kc * 512 : (kc + 1) * 512],
                                pss[:],
                                mybir.ActivationFunctionType.Exp,
                                scale=SCALE,
                                accum_out=rs[:, kc : kc + 1],
                            )

                    attnT = attnT_p.tile([128, SEQ // 128, 128], F32R, tag="attnT")
                    for kc in range(ncs):
                        pst = ps_t3.tile([128, 512], F32, tag="ps_t3")
                        for t in range(4):
                            nc.tensor.matmul(
                                pst[:, t * 128 : (t + 1) * 128],
                                attn[:, (kc * 4 + t) * 128 : (kc * 4 + t + 1) * 128],
                                ident[:],
                                is_transpose=True,
                                start=(t == 0),
                                stop=(t == 3),
                            )
                        nc.vector.tensor_copy(attnT[:, kc * 4 : kc * 4 + 4, :], pst[:])

                    nkt = 4 * ncs
                    pc1 = ps_c1.tile([128, 512], F32, tag="ps_c1")
                    pc2 = ps_c2.tile([128, 256], F32, tag="ps_c2")
                    for ktile in range(nkt):
                        nc.tensor.matmul(
                            pc1[:],
                            attnT[:, ktile, :],
                            vt[:, ktile, 0:512],
                            start=(ktile == 0),
                            stop=(ktile == nkt - 1),
                        )
                    for ktile in range(nkt):
                        nc.tensor.matmul(
                            pc2[:],
                            attnT[:, ktile, :],
                            vt[:, ktile, 512:768],
                            start=(ktile == 0),
                            stop=(ktile == nkt - 1),
                        )

                    rsum = small_p.tile([128, 1], F32, tag="rsum")
                    nc.vector.reduce_sum(
                        rsum[:], rs[:, 0:ncs], axis=mybir.AxisListType.X
                    )
                    rinv = small_p.tile([128, 1], F32, tag="rinv")
                    nc.vector.reciprocal(rinv[:], rsum[:])

                    ctx_sb = ctx_p.tile([128, D], F32, tag="ctxs")
                    nc.vector.tensor_mul(
                        ctx_sb[:, 0:512], pc1[:], rinv[:].to_broadcast((128, 512))
                    )
                    nc.vector.tensor_mul(
                        ctx_sb[:, 512:768], pc2[:], rinv[:].to_broadcast((128, 256))
                    )
                    nc.sync.dma_start(out_d[lt * 128 : (lt + 1) * 128, :], ctx_sb[:])

    nc.compile()
    return nc


def _strip_variant(v):
    """Within-chunk causal mask for a diagonal chunk of residue v = g mod 4:
    allow key jj (0..511) for row i iff jj <= 128*v + i."""
    i = np.arange(128)[:, None]
    jj = np.arange(512)[None, :]
    return np.where(jj <= 128 * v + i, 0.0, NEG).astype(np.float32)


def kernel(x, Wq, Wk, Wv):
    if "nc" not in _CACHE:
        _CACHE["nc"] = _build()
    nc = _CACHE["nc"]

    x = np.ascontiguousarray(x, dtype=np.float32)
    wqt = np.ascontiguousarray(np.asarray(Wq, dtype=np.float32).T)
    wkt = np.ascontiguousarray(np.asarray(Wk, dtype=np.float32).T)
    wvt = np.ascontiguousarray(np.asarray(Wv, dtype=np.float32).T)

    in_maps = []
    for c in range(8):
        b, h = c // 2, c % 2
        xb = x[b]
        # own query rows: global q-tiles 2*lt + h
        own = np.concatenate(
            [xb[(2 * lt + h) * 128 : (2 * lt + h + 1) * 128] for lt in range(LT)],
            axis=0,
        )
        # strip windows: lt%2==0 -> variant h; lt%2==1 -> variant 2+h
        strip = np.concatenate([_strip_variant(h), _strip_variant(2 + h)], axis=1)
        in_maps.append(
            {
                "xt": np.ascontiguousarray(xb.T),
                "xqt": np.ascontiguousarray(own.T),
                "wqt": wqt,
                "wkt": wkt,
                "wvt": wvt,
                "strip": np.ascontiguousarray(strip),
            }
        )

    res = run_bass_kernel_spmd(
        nc,
        in_maps,
        list(range(8)),
        trace=bool(int(os.environ.get("KERNEL_TRACE", "0"))),
    )
    _CACHE["last_results"] = res

    out = np.empty((BATCH, SEQ, D), np.float32)
    for c in range(8):
        b, h = c // 2, c % 2
        o = res.results[c]["out"]
        for lt in range(LT):
            out[b, (2 * lt + h) * 128 : (2 * lt + h + 1) * 128] = o[
                lt * 128 : (lt + 1) * 128
            ]
    return out



# revision 5
# speedup vs baseline: 1.5739x; 1.5739x over previous
"""Causal single-head attention on 8 TRN2 NeuronCores — fp8/bf16 edition.

Problem: x [4, 2048, 768] f32; Wq/Wk/Wv [768, 768] f32 (torch Linear layout).
  q/k/v = x @ W.T ; scores = q k^T causal-masked; attn = softmax(scores/sqrt(768));
  out = attn @ v.

Sharding: core c -> batch b = c//2, half h = c%2. Core h owns global q-tiles
{2lt+h}, grouped into 4 PAIRS: pair p = global tiles (4p+h, 4p+2+h). The
uniform SPMD program processes key-tiles 0..4p+3 for pair p on every core;
which entries are causally masked is pure per-core DATA (the strip input).

Precision strategy (tolerance 2e-2; fp8 DoubleRow matmuls are 4x f32r rate,
bf16 is 2x, in the grading cost model):
  - All weights are pre-scaled by 32 on the host so that both fp8(32W) and
    the fp8 residual fp8(32W - fp8(32W)) sit well above e4m3's minimum
    subnormal (2^-9) — unscaled, |W|<=0.036 makes the residual term flush
    to zero. Projections run as 3-term fp8 DoubleRow hi/lo splits
    (x_hi@W_hi + x_hi@W_lo + x_lo@W_hi), giving ~bf16 accuracy at 75% of
    bf16 PE cost. The x32 scaling cancels: q,k stay scaled (32q, 32k; the
    1024x on scores folds into the exp scale constant), v is unscaled by
    1/32 during psum evacuation (a scaled copy, same cost).
  - QK^T scores: fp8 DoubleRow on fp8-cast 32q/32k (|32q| <= ~130 < 240).
    The only score noise is the fp8 cast; softmax normalization cancels
    common-mode and peaked rows are insensitive. Measured ~1.3e-2.
  - attn@V context: bf16 (early causal rows copy v rows verbatim), with a
    ones-column appended to V so the softmax denominator falls out of the
    same matmul (exact normalization even after quantization).

Scores are computed TRANSPOSED (S^T = K Q^T with d on the contraction
partitions): the exp result in [key, query] layout feeds the context matmul
directly as the stationary operand — no PE transposes at all. Causal masking
is done ON THE PE: the last accumulation step of a diagonal tile's psum group
is matmul(diag(-1e30), strip01), adding -1e30 wherever strip==1.

Schedule: phase order K -> V -> Q -> attention. K+V are PE-heavy but only
need ~5.4MB of input, so the DMA-heavy/PE-light Q inputs stream in their
shadow. Weights and x-chunks are split into per-ko-pair tiles so the first
matmul of each phase starts after ~0.6MB. The attention inner loop is
software-pipelined (scores/exp run 2 key-tiles ahead of the context
accumulation) so the PE never waits on the Activation engine.
"""

import os
import sys
from contextlib import ExitStack

import numpy as np

for _p in ("/opt/trn_rl_repo", "/root/.axon_site/_ro/trn_rl_repo"):
    if os.path.isdir(_p) and _p not in sys.path:
        sys.path.append(_p)

import ml_dtypes  # noqa: E402

import concourse.mybir as mybir  # noqa: E402
import concourse.tile as tile  # noqa: E402
from concourse import bacc  # noqa: E402
from concourse.bass_utils import run_bass_kernel_spmd  # noqa: E402

F32 = mybir.dt.float32
BF16 = mybir.dt.bfloat16
FP8 = mybir.dt.float8e4
NP_FP8 = ml_dtypes.float8_e4m3
NP_BF16 = ml_dtypes.bfloat16
DR = mybir.MatmulPerfMode.DoubleRow
EXP = mybir.ActivationFunctionType.Exp

BATCH = 4
SEQ = 2048
D = 768
DK = D // 128  # 6 contraction k-tiles; 3 DoubleRow pairs
NQ = 1024  # query rows per core
WS = 32.0  # host-side weight pre-scale
ESCALE = float(1.0 / (np.sqrt(np.float32(D)) * WS * WS))
IVS = float(1.0 / WS)
NEG = -1e30

_CACHE = {}


def _build():
    nc = bacc.Bacc("TRN2", target_bir_lowering=False, debug=False, num_devices=8)
    xt_d = nc.declare_dram_parameter("xt", [D, SEQ], FP8, isOutput=False)
    xlo_d = nc.declare_dram_parameter("xlo", [D, SEQ], FP8, isOutput=False)
    xqt_d = nc.declare_dram_parameter("xqt", [D, NQ], FP8, isOutput=False)
    xqlo_d = nc.declare_dram_parameter("xqlo", [D, NQ], FP8, isOutput=False)
    wq_d = nc.declare_dram_parameter("wq", [D, D], FP8, isOutput=False)
    wql_d = nc.declare_dram_parameter("wql", [D, D], FP8, isOutput=False)
    wk_d = nc.declare_dram_parameter("wk", [D, D], FP8, isOutput=False)
    wkl_d = nc.declare_dram_parameter("wkl", [D, D], FP8, isOutput=False)
    wvh_d = nc.declare_dram_parameter("wvh", [D, D], FP8, isOutput=False)
    wvl_d = nc.declare_dram_parameter("wvl", [D, D], FP8, isOutput=False)
    strip_d = nc.declare_dram_parameter("strip", [128, 1024], BF16, isOutput=False)
    negd_d = nc.declare_dram_parameter("negd", [128, 128], BF16, isOutput=False)
    out_d = nc.declare_dram_parameter("out", [NQ, D], F32, isOutput=True)

    # Input DMAs round-robin across the SP and Pool DGE queues (Act/DVE stay
    # free for psum evacuation). The DMA engines serialize, so EMISSION ORDER
    # is the prefetch schedule.
    _dma_i = [0]

    def dma_in(dst, src):
        eng = (nc.sync, nc.gpsimd)[_dma_i[0] % 2]
        eng.dma_start(dst, src)
        _dma_i[0] += 1

    _dma_o = [0]

    def dma_out(dst, src):
        eng = (nc.sync, nc.gpsimd)[_dma_o[0] % 2]
        eng.dma_start(dst, src)
        _dma_o[0] += 1

    # Psum evacuations alternate DVE / Act; V variant fuses the 1/32 unscale.
    _evac_i = [0]

    def evac(dst, src, scale=None):
        if _evac_i[0] % 2 == 0:
            if scale is None:
                nc.vector.tensor_copy(dst, src)
            else:
                nc.vector.tensor_scalar_mul(dst, src, scale)
        else:
            if scale is None:
                nc.scalar.copy(dst, src)
            else:
                nc.scalar.mul(dst, src, scale)
        _evac_i[0] += 1

    def rearr(dram_slice):
        return dram_slice.rearrange("(ko p) s -> p ko s", p=128)

    def wload(pool, name, dram):
        """Weight matrix as 3 per-ko-pair tiles [128, 2, D]; DMA each."""
        ts = []
        for j in range(3):
            t = pool.tile([128, 2, D], FP8, name=f"{name}{j}")
            dma_in(t[:], rearr(dram[j * 256 : (j + 1) * 256, :]))
            ts.append(t)
        return ts

    def xchunk(pool, name, dram, sc, tag=False):
        """x^T 512-col chunk as 3 per-ko-pair tiles [128, 2, 512]."""
        ts = []
        for j in range(3):
            kw = {"tag": f"{name}{j}"} if tag else {"name": f"{name}{j}"}
            t = pool.tile([128, 2, 512], FP8, **kw)
            dma_in(
                t[:],
                rearr(dram[j * 256 : (j + 1) * 256, sc * 512 : sc * 512 + 512]),
            )
            ts.append(t)
        return ts

    with tile.TileContext(nc) as tc, ExitStack() as ctx:
        persist = ctx.enter_context(tc.tile_pool(name="persist", bufs=1))

        qth = [persist.tile([128, DK, 512], FP8, name=f"qth{i}") for i in range(2)]
        kt = persist.tile([128, DK, SEQ], FP8)  # K^T resident (fp8 cast, 32-scaled)
        vtc = [persist.tile([128, 4, 769], BF16, name=f"vtc{i}") for i in range(4)]
        strip = persist.tile([128, 1024], BF16)
        negd = persist.tile([128, 128], BF16)

        with ExitStack() as pw:
            xp = pw.enter_context(tc.tile_pool(name="xp", bufs=1))
            wkp = pw.enter_context(tc.tile_pool(name="wkp", bufs=1))
            wvp = pw.enter_context(tc.tile_pool(name="wvp", bufs=1))
            wqp = pw.enter_context(tc.tile_pool(name="wqp", bufs=1))
            xqp = pw.enter_context(tc.tile_pool(name="xqp", bufs=1))

            # ---------------- Phase K: K^T projection ----------------
            with ExitStack() as pK:
                psk = pK.enter_context(tc.tile_pool(name="psk", bufs=3, space="PSUM"))
                wk = wload(wkp, "wk", wk_d)
                wkl = wload(wkp, "wkl", wkl_d)
                xt8c, xlo8c = [], []
                for sc in range(4):
                    xt8c.append(xchunk(xp, f"xt8c{sc}_", xt_d, sc))
                    xlo8c.append(xchunk(xp, f"xlo8c{sc}_", xlo_d, sc))
                    if sc == 1:
                        wvh = wload(wvp, "wvh", wvh_d)
                        wvl = wload(wvp, "wvl", wvl_d)
                    for oo in range(DK):
                        pk = psk.tile([128, 512], F32, tag="psk")
                        n = 0
                        for wop, xop in (
                            (wk, xt8c[sc]),
                            (wkl, xt8c[sc]),
                            (wk, xlo8c[sc]),
                        ):
                            for j in range(3):
                                nc.tensor.matmul(
                                    pk[:],
                                    wop[j][:, :, oo * 128 : oo * 128 + 128],
                                    xop[j][:],
                                    perf_mode=DR,
                                    start=(n == 0),
                                    stop=(n == 8),
                                )
                                n += 1
                        evac(kt[:, oo, sc * 512 : sc * 512 + 512], pk[:])

            # ---------------- Phase V: V projection ----------------
            with ExitStack() as pV:
                psv = pV.enter_context(tc.tile_pool(name="psv", bufs=3, space="PSUM"))
                wq = wload(wqp, "wq", wq_d)
                wql = wload(wqp, "wql", wql_d)
                xqc = [None, None]
                xqlc = [None, None]
                for sc in range(4):
                    if sc < 2:
                        xqc[sc] = xchunk(xqp, f"xqc{sc}_", xqt_d, sc)
                        xqlc[sc] = xchunk(xqp, f"xqlc{sc}_", xqlo_d, sc)
                    if sc == 2:
                        dma_in(strip[:], strip_d[:])
                        dma_in(negd[:], negd_d[:])
                    nc.vector.memset(vtc[sc][:, :, 768:769], 1.0)
                    for st in range(4):
                        for oc in range(2):
                            pv = psv.tile([128, 384], F32, tag="psv")
                            n = 0
                            for xop, wop in (
                                (xt8c[sc], wvh),
                                (xt8c[sc], wvl),
                                (xlo8c[sc], wvh),
                            ):
                                for j in range(3):
                                    nc.tensor.matmul(
                                        pv[:],
                                        xop[j][:, :, st * 128 : st * 128 + 128],
                                        wop[j][:, :, oc * 384 : oc * 384 + 384],
                                        perf_mode=DR,
                                        start=(n == 0),
                                        stop=(n == 8),
                                    )
                                    n += 1
                            evac(vtc[sc][:, st, oc * 384 : oc * 384 + 384], pv[:], scale=IVS)

            # ---------------- Phase Q: Q^T projection ----------------
            with ExitStack() as pQ:
                psq = pQ.enter_context(tc.tile_pool(name="psq", bufs=3, space="PSUM"))
                for sc in range(2):
                    for oo in range(DK):
                        pq = psq.tile([128, 512], F32, tag="psq")
                        n = 0
                        for wop, xop in (
                            (wq, xqc[sc]),
                            (wql, xqc[sc]),
                            (wq, xqlc[sc]),
                        ):
                            for j in range(3):
                                nc.tensor.matmul(
                                    pq[:],
                                    wop[j][:, :, oo * 128 : oo * 128 + 128],
                                    xop[j][:],
                                    perf_mode=DR,
                                    start=(n == 0),
                                    stop=(n == 8),
                                )
                                n += 1
                        evac(qth[sc][:, oo, :], pq[:])

        # ---------------- Phase A: attention (S^T scheme) ----------------
        with ExitStack() as pA:
            pss_p = pA.enter_context(tc.tile_pool(name="pss", bufs=3, space="PSUM"))
            pc1a_p = pA.enter_context(tc.tile_pool(name="pc1a", bufs=1, space="PSUM"))
            pc2a_p = pA.enter_context(tc.tile_pool(name="pc2a", bufs=1, space="PSUM"))
            pc1b_p = pA.enter_context(tc.tile_pool(name="pc1b", bufs=1, space="PSUM"))
            pc2b_p = pA.enter_context(tc.tile_pool(name="pc2b", bufs=1, space="PSUM"))
            attn_p = pA.enter_context(tc.tile_pool(name="attn", bufs=4))
            out_p = pA.enter_context(tc.tile_pool(name="outp", bufs=2))
            small_p = pA.enter_context(tc.tile_pool(name="small", bufs=2))

            LOOK = 2  # scores/exp run this many key-tiles ahead of context

            for p in range(4):
                pc1 = [
                    pc1a_p.tile([128, 512], F32, name="pc1a"),
                    pc1b_p.tile([128, 512], F32, name="pc1b"),
                ]
                pc2 = [
                    pc2a_p.tile([128, 257], F32, name="pc2a"),
                    pc2b_p.tile([128, 257], F32, name="pc2b"),
                ]
                nkt = [4 * p + 2, 4 * p + 4]  # ctx key-tile count per q-block
                N = 4 * p + 4
                ats = {}
                for it in range(N + LOOK):
                    if it < N:
                        pss = pss_p.tile([128, 256], F32, tag="pss")
                        di = it - 4 * p
                        for j in range(3):
                            nc.tensor.matmul(
                                pss[:],
                                kt[:, 2 * j : 2 * j + 2, it * 128 : it * 128 + 128],
                                qth[p // 2][
                                    :, 2 * j : 2 * j + 2,
                                    (p % 2) * 256 : (p % 2) * 256 + 256,
                                ],
                                perf_mode=DR,
                                start=(j == 0),
                                stop=(j == 2 and di < 0),
                            )
                        if di >= 0:
                            # causal mask on the PE: psum += -1e30 * strip01
                            nc.tensor.matmul(
                                pss[:],
                                negd[:],
                                strip[:, di * 256 : di * 256 + 256],
                                start=False,
                                stop=True,
                            )
                        at = attn_p.tile([128, 256], BF16, tag="attn")
                        nc.scalar.activation(at[:], pss[:], EXP, scale=ESCALE)
                        ats[it] = at
                    kc = it - LOOK
                    if kc < 0:
                        continue
                    at = ats[kc]
                    for blk in range(2):
                        if kc < nkt[blk]:
                            last = kc == nkt[blk] - 1
                            nc.tensor.matmul(
                                pc1[blk][:],
                                at[:, blk * 128 : blk * 128 + 128],
                                vtc[kc // 4][:, kc % 4, 0:512],
                                start=(kc == 0),
                                stop=last,
                            )
                            nc.tensor.matmul(
                                pc2[blk][:],
                                at[:, blk * 128 : blk * 128 + 128],
                                vtc[kc // 4][:, kc % 4, 512:769],
                                start=(kc == 0),
                                stop=last,
                            )
                            if last:
                                # normalize + store this q-block immediately
                                rinv = small_p.tile([128, 1], F32, tag="rinv")
                                nc.vector.reciprocal(rinv[:], pc2[blk][:, 256:257])
                                osb = out_p.tile([128, D], F32, tag="osb")
                                nc.vector.tensor_mul(
                                    osb[:, 0:512],
                                    pc1[blk][:],
                                    rinv[:].to_broadcast((128, 512)),
                                )
                                nc.vector.tensor_mul(
                                    osb[:, 512:768],
                                    pc2[blk][:, 0:256],
                                    rinv[:].to_broadcast((128, 256)),
                                )
                                r = (2 * p + blk) * 128
                                dma_out(out_d[r : r + 128, :], osb[:])

    nc.compile()
    return nc


def _make_strip(h):
    """[128, 1024] 0/1 mask; block i (256 wide) is added (via -1e30) to the
    S^T psum at diagonal offset i = kt - 4p. Layout [key-row, query-col]."""
    tri = (np.arange(128)[:, None] > np.arange(128)[None, :]).astype(np.float32)
    ones = np.ones((128, 128), np.float32)
    zeros = np.zeros((128, 128), np.float32)
    if h == 0:
        blocks = [(tri, zeros), (ones, zeros), (zeros, tri), (zeros, ones)]
    else:
        blocks = [(zeros, zeros), (tri, zeros), (zeros, zeros), (zeros, tri)]
    return np.concatenate([np.concatenate(b, axis=1) for b in blocks], axis=1)


def _hi_lo(a):
    hi = a.astype(NP_FP8)
    lo = (a - hi.astype(np.float32)).astype(NP_FP8)
    return hi, lo


def kernel(x, Wq, Wk, Wv):
    if "nc" not in _CACHE:
        _CACHE["nc"] = _build()
    nc = _CACHE["nc"]

    x = np.ascontiguousarray(x, dtype=np.float32)
    wq8, wql8 = _hi_lo(WS * np.asarray(Wq, dtype=np.float32).T)
    wk8, wkl8 = _hi_lo(WS * np.asarray(Wk, dtype=np.float32).T)
    wvh8, wvl8 = _hi_lo(WS * np.asarray(Wv, dtype=np.float32).T)
    negd = (NEG * np.eye(128, dtype=np.float32)).astype(NP_BF16)

    in_maps = []
    for c in range(8):
        b, h = c // 2, c % 2
        xbt = np.ascontiguousarray(x[b].T)  # [768, 2048]
        xt8, xlo8 = _hi_lo(xbt)
        # own query columns: pairs p -> global tiles (4p+h, 4p+2+h)
        cols = []
        for p in range(4):
            for g in (4 * p + h, 4 * p + 2 + h):
                cols.append(xbt[:, g * 128 : (g + 1) * 128])
        xqf = np.ascontiguousarray(np.concatenate(cols, axis=1))
        xqt8, xqlo8 = _hi_lo(xqf)
        in_maps.append(
            {
                "xt": xt8,
                "xlo": xlo8,
                "xqt": xqt8,
                "xqlo": xqlo8,
                "wq": wq8,
                "wql": wql8,
                "wk": wk8,
                "wkl": wkl8,
                "wvh": wvh8,
                "wvl": wvl8,
                "strip": _make_strip(h).astype(NP_BF16),
                "negd": negd,
            }
        )

    res = run_bass_kernel_spmd(
        nc,
        in_maps,
        list(range(8)),
        trace=bool(int(os.environ.get("KERNEL_TRACE", "0"))),
    )
    _CACHE["last_results"] = res

    out = np.empty((BATCH, SEQ, D), np.float32)
    for c in range(8):
        b, h = c // 2, c % 2
        o = res.results[c]["out"]
        for p in range(4):
            for blk, g in enumerate((4 * p + h, 4 * p + 2 + h)):
                out[b, g * 128 : (g + 1) * 128] = o[
                    (2 * p + blk) * 128 : (2 * p + blk + 1) * 128
                ]
    return out


# revision 9
# speedup vs baseline: 1.5746x; 1.0004x over previous
"""Causal single-head attention on 8 TRN2 NeuronCores — fp8/bf16 edition.

Problem: x [4, 2048, 768] f32; Wq/Wk/Wv [768, 768] f32 (torch Linear layout).
  q/k/v = x @ W.T ; scores = q k^T causal-masked; attn = softmax(scores/sqrt(768));
  out = attn @ v.

Sharding: core c -> batch b = c//2, half h = c%2. Core h owns global q-tiles
{2lt+h}, grouped into 4 PAIRS: pair p = global tiles (4p+h, 4p+2+h). The
uniform SPMD program processes key-tiles 0..4p+3 for pair p on every core;
which entries are causally masked is pure per-core DATA (the strip input).

Precision strategy (tolerance 2e-2; fp8 DoubleRow matmuls are 4x f32r rate,
bf16 is 2x, in the grading cost model):
  - All weights are pre-scaled by 32 on the host so that both fp8(32W) and
    the fp8 residual fp8(32W - fp8(32W)) sit well above e4m3's minimum
    subnormal (2^-9) — unscaled, |W|<=0.036 makes the residual term flush
    to zero. Projections run as 3-term fp8 DoubleRow hi/lo splits
    (x_hi@W_hi + x_hi@W_lo + x_lo@W_hi), giving ~bf16 accuracy at 75% of
    bf16 PE cost. The x32 scaling cancels: q,k stay scaled (32q, 32k; the
    1024x on scores folds into the exp scale constant), v is unscaled by
    1/32 during psum evacuation (a scaled copy, same cost).
  - QK^T scores: fp8 DoubleRow on fp8-cast 32q/32k (|32q| <= ~130 < 240).
    The only score noise is the fp8 cast; softmax normalization cancels
    common-mode and peaked rows are insensitive. Measured ~1.3e-2.
  - attn@V context: bf16 (early causal rows copy v rows verbatim), with a
    ones-column appended to V so the softmax denominator falls out of the
    same matmul (exact normalization even after quantization).

Scores are computed TRANSPOSED (S^T = K Q^T with d on the contraction
partitions): the exp result in [key, query] layout feeds the context matmul
directly as the stationary operand — no PE transposes at all. Causal masking
is done ON THE PE: the last accumulation step of a diagonal tile's psum group
is matmul(diag(-1e30), strip01), adding -1e30 wherever strip==1.

Schedule: phase order K -> V -> Q -> attention. K+V are PE-heavy but only
need ~5.4MB of input, so the DMA-heavy/PE-light Q inputs stream in their
shadow. Weights and x-chunks are split into per-ko-pair tiles so the first
matmul of each phase starts after ~0.6MB. The attention inner loop is
software-pipelined (scores/exp run 2 key-tiles ahead of the context
accumulation) so the PE never waits on the Activation engine.
"""

import os
import sys
from contextlib import ExitStack

import numpy as np

for _p in ("/opt/trn_rl_repo", "/root/.axon_site/_ro/trn_rl_repo"):
    if os.path.isdir(_p) and _p not in sys.path:
        sys.path.append(_p)

import ml_dtypes  # noqa: E402

import concourse.mybir as mybir  # noqa: E402
import concourse.tile as tile  # noqa: E402
from concourse import bacc  # noqa: E402
from concourse.bass_utils import run_bass_kernel_spmd  # noqa: E402

F32 = mybir.dt.float32
BF16 = mybir.dt.bfloat16
FP8 = mybir.dt.float8e4
NP_FP8 = ml_dtypes.float8_e4m3
NP_BF16 = ml_dtypes.bfloat16
DR = mybir.MatmulPerfMode.DoubleRow
EXP = mybir.ActivationFunctionType.Exp

BATCH = 4
SEQ = 2048
D = 768
DK = D // 128  # 6 contraction k-tiles; 3 DoubleRow pairs
NQ = 1024  # query rows per core
WS = 32.0  # host-side weight pre-scale
ESCALE = float(1.0 / (np.sqrt(np.float32(D)) * WS * WS))
IVS = float(1.0 / WS)
NEG = -1e30

_CACHE = {}


def _build():
    nc = bacc.Bacc("TRN2", target_bir_lowering=False, debug=False, num_devices=8)
    xt_d = nc.declare_dram_parameter("xt", [D, SEQ], FP8, isOutput=False)
    xlo_d = nc.declare_dram_parameter("xlo", [D, SEQ], FP8, isOutput=False)
    xqt_d = nc.declare_dram_parameter("xqt", [D, NQ], FP8, isOutput=False)
    xqlo_d = nc.declare_dram_parameter("xqlo", [D, NQ], FP8, isOutput=False)
    wq_d = nc.declare_dram_parameter("wq", [D, D], FP8, isOutput=False)
    wql_d = nc.declare_dram_parameter("wql", [D, D], FP8, isOutput=False)
    wk_d = nc.declare_dram_parameter("wk", [D, D], FP8, isOutput=False)
    wkl_d = nc.declare_dram_parameter("wkl", [D, D], FP8, isOutput=False)
    wvh_d = nc.declare_dram_parameter("wvh", [D, D], FP8, isOutput=False)
    wvl_d = nc.declare_dram_parameter("wvl", [D, D], FP8, isOutput=False)
    strip_d = nc.declare_dram_parameter("strip", [128, 1024], BF16, isOutput=False)
    negd_d = nc.declare_dram_parameter("negd", [128, 128], BF16, isOutput=False)
    out_d = nc.declare_dram_parameter("out", [NQ, D], F32, isOutput=True)

    # Input DMAs round-robin across the SP and Pool DGE queues (Act/DVE stay
    # free for psum evacuation). The DMA engines serialize, so EMISSION ORDER
    # is the prefetch schedule.
    _dma_i = [0]

    def dma_in(dst, src):
        eng = (nc.sync, nc.gpsimd)[_dma_i[0] % 2]
        eng.dma_start(dst, src)
        _dma_i[0] += 1

    _dma_o = [0]

    def dma_out(dst, src):
        eng = (nc.sync, nc.gpsimd)[_dma_o[0] % 2]
        eng.dma_start(dst, src)
        _dma_o[0] += 1

    # Psum evacuations alternate DVE / Act; V variant fuses the 1/32 unscale.
    _evac_i = [0]

    def evac(dst, src, scale=None):
        if _evac_i[0] % 2 == 0:
            if scale is None:
                nc.vector.tensor_copy(dst, src)
            else:
                nc.vector.tensor_scalar_mul(dst, src, scale)
        else:
            if scale is None:
                nc.scalar.copy(dst, src)
            else:
                nc.scalar.mul(dst, src, scale)
        _evac_i[0] += 1

    def rearr(dram_slice):
        return dram_slice.rearrange("(ko p) s -> p ko s", p=128)

    def wload(pool, name, dram):
        """Weight matrix as 3 per-ko-pair tiles [128, 2, D]; DMA each."""
        ts = []
        for j in range(3):
            t = pool.tile([128, 2, D], FP8, name=f"{name}{j}")
            dma_in(t[:], rearr(dram[j * 256 : (j + 1) * 256, :]))
            ts.append(t)
        return ts

    def xchunk(pool, name, dram, sc, tag=False):
        """x^T 512-col chunk as 3 per-ko-pair tiles [128, 2, 512]."""
        ts = []
        for j in range(3):
            kw = {"tag": f"{name}{j}"} if tag else {"name": f"{name}{j}"}
            t = pool.tile([128, 2, 512], FP8, **kw)
            dma_in(
                t[:],
                rearr(dram[j * 256 : (j + 1) * 256, sc * 512 : sc * 512 + 512]),
            )
            ts.append(t)
        return ts

    with tile.TileContext(nc) as tc, ExitStack() as ctx:
        persist = ctx.enter_context(tc.tile_pool(name="persist", bufs=1))

        qth = [persist.tile([128, DK, 512], FP8, name=f"qth{i}") for i in range(2)]
        kt = persist.tile([128, DK, SEQ], FP8)  # K^T resident (fp8 cast, 32-scaled)
        vtc = [persist.tile([128, 4, 769], BF16, name=f"vtc{i}") for i in range(4)]
        strip = persist.tile([128, 1024], BF16)
        negd = persist.tile([128, 128], BF16)

        with ExitStack() as pw:
            xp = pw.enter_context(tc.tile_pool(name="xp", bufs=1))
            wkp = pw.enter_context(tc.tile_pool(name="wkp", bufs=1))
            wvp = pw.enter_context(tc.tile_pool(name="wvp", bufs=1))
            wqp = pw.enter_context(tc.tile_pool(name="wqp", bufs=1))
            xqp = pw.enter_context(tc.tile_pool(name="xqp", bufs=1))

            # ---------------- Phase K: K^T projection ----------------
            with ExitStack() as pK:
                psk = pK.enter_context(tc.tile_pool(name="psk", bufs=3, space="PSUM"))
                # j-interleaved emission: the first psum group's operand tiles
                # arrive in matmul order, so the PE starts after ~0.6MB.
                wk = [wkp.tile([128, 2, D], FP8, name=f"wk{j}") for j in range(3)]
                wkl = [wkp.tile([128, 2, D], FP8, name=f"wkl{j}") for j in range(3)]
                xt8c = [
                    [xp.tile([128, 2, 512], FP8, name=f"xt8c{sc}_{j}") for j in range(3)]
                    for sc in range(4)
                ]
                xlo8c = [
                    [xp.tile([128, 2, 512], FP8, name=f"xlo8c{sc}_{j}") for j in range(3)]
                    for sc in range(4)
                ]
                for j in range(3):
                    dma_in(wk[j][:], rearr(wk_d[j * 256 : (j + 1) * 256, :]))
                    dma_in(xt8c[0][j][:], rearr(xt_d[j * 256 : (j + 1) * 256, 0:512]))
                    dma_in(wkl[j][:], rearr(wkl_d[j * 256 : (j + 1) * 256, :]))
                    dma_in(xlo8c[0][j][:], rearr(xlo_d[j * 256 : (j + 1) * 256, 0:512]))
                for sc in range(4):
                    if sc > 0:
                        for j in range(3):
                            dma_in(
                                xt8c[sc][j][:],
                                rearr(xt_d[j * 256 : (j + 1) * 256, sc * 512 : sc * 512 + 512]),
                            )
                            dma_in(
                                xlo8c[sc][j][:],
                                rearr(xlo_d[j * 256 : (j + 1) * 256, sc * 512 : sc * 512 + 512]),
                            )
                    if sc == 1:
                        wvh = wload(wvp, "wvh", wvh_d)
                        wvl = wload(wvp, "wvl", wvl_d)
                    for oo in range(DK):
                        pk = psk.tile([128, 512], F32, tag="psk")
                        n = 0
                        for wop, xop in (
                            (wk, xt8c[sc]),
                            (wkl, xt8c[sc]),
                            (wk, xlo8c[sc]),
                        ):
                            for j in range(3):
                                nc.tensor.matmul(
                                    pk[:],
                                    wop[j][:, :, oo * 128 : oo * 128 + 128],
                                    xop[j][:],
                                    perf_mode=DR,
                                    start=(n == 0),
                                    stop=(n == 8),
                                )
                                n += 1
                        evac(kt[:, oo, sc * 512 : sc * 512 + 512], pk[:])

            # ---------------- Phase V: V projection ----------------
            with ExitStack() as pV:
                psv = pV.enter_context(tc.tile_pool(name="psv", bufs=3, space="PSUM"))
                wq = wload(wqp, "wq", wq_d)
                wql = wload(wqp, "wql", wql_d)
                xqc = [None, None]
                xqlc = [None, None]
                for sc in range(4):
                    if sc < 2:
                        xqc[sc] = xchunk(xqp, f"xqc{sc}_", xqt_d, sc)
                        xqlc[sc] = xchunk(xqp, f"xqlc{sc}_", xqlo_d, sc)
                    if sc == 2:
                        dma_in(strip[:], strip_d[:])
                        dma_in(negd[:], negd_d[:])
                    nc.vector.memset(vtc[sc][:, :, 768:769], 1.0)
                    for st in range(4):
                        for oc in range(2):
                            pv = psv.tile([128, 384], F32, tag="psv")
                            n = 0
                            for xop, wop in (
                                (xt8c[sc], wvh),
                                (xt8c[sc], wvl),
                                (xlo8c[sc], wvh),
                            ):
                                for j in range(3):
                                    nc.tensor.matmul(
                                        pv[:],
                                        xop[j][:, :, st * 128 : st * 128 + 128],
                                        wop[j][:, :, oc * 384 : oc * 384 + 384],
                                        perf_mode=DR,
                                        start=(n == 0),
                                        stop=(n == 8),
                                    )
                                    n += 1
                            evac(vtc[sc][:, st, oc * 384 : oc * 384 + 384], pv[:], scale=IVS)

            # ---------------- Phase Q: Q^T projection ----------------
            with ExitStack() as pQ:
                psq = pQ.enter_context(tc.tile_pool(name="psq", bufs=3, space="PSUM"))
                for sc in range(2):
                    for oo in range(DK):
                        pq = psq.tile([128, 512], F32, tag="psq")
                        n = 0
                        for wop, xop in (
                            (wq, xqc[sc]),
                            (wql, xqc[sc]),
                            (wq, xqlc[sc]),
                        ):
                            for j in range(3):
                                nc.tensor.matmul(
                                    pq[:],
                                    wop[j][:, :, oo * 128 : oo * 128 + 128],
                                    xop[j][:],
                                    perf_mode=DR,
                                    start=(n == 0),
                                    stop=(n == 8),
                                )
                                n += 1
                        evac(qth[sc][:, oo, :], pq[:])

        # ---------------- Phase A: attention (S^T scheme) ----------------
        with ExitStack() as pA:
            pss_p = pA.enter_context(tc.tile_pool(name="pss", bufs=2, space="PSUM"))
            pc1a_p = pA.enter_context(tc.tile_pool(name="pc1a", bufs=2, space="PSUM"))
            pc2a_p = pA.enter_context(tc.tile_pool(name="pc2a", bufs=1, space="PSUM"))
            pc1b_p = pA.enter_context(tc.tile_pool(name="pc1b", bufs=2, space="PSUM"))
            pc2b_p = pA.enter_context(tc.tile_pool(name="pc2b", bufs=1, space="PSUM"))
            attn_p = pA.enter_context(tc.tile_pool(name="attn", bufs=4))
            out_p = pA.enter_context(tc.tile_pool(name="outp", bufs=2))
            small_p = pA.enter_context(tc.tile_pool(name="small", bufs=2))

            LOOK = 2  # scores/exp run this many key-tiles ahead of context

            for p in range(4):
                pc1 = [
                    pc1a_p.tile([128, 512], F32, name="pc1a"),
                    pc1b_p.tile([128, 512], F32, name="pc1b"),
                ]
                pc2 = [
                    pc2a_p.tile([128, 257], F32, name="pc2a"),
                    pc2b_p.tile([128, 257], F32, name="pc2b"),
                ]
                nkt = [4 * p + 2, 4 * p + 4]  # ctx key-tile count per q-block
                N = 4 * p + 4
                ats = {}
                for it in range(N + LOOK):
                    if it < N:
                        pss = pss_p.tile([128, 256], F32, tag="pss")
                        di = it - 4 * p
                        for j in range(3):
                            nc.tensor.matmul(
                                pss[:],
                                kt[:, 2 * j : 2 * j + 2, it * 128 : it * 128 + 128],
                                qth[p // 2][
                                    :, 2 * j : 2 * j + 2,
                                    (p % 2) * 256 : (p % 2) * 256 + 256,
                                ],
                                perf_mode=DR,
                                start=(j == 0),
                                stop=(j == 2 and di < 0),
                            )
                        if di >= 0:
                            # causal mask on the PE: psum += -1e30 * strip01
                            nc.tensor.matmul(
                                pss[:],
                                negd[:],
                                strip[:, di * 256 : di * 256 + 256],
                                start=False,
                                stop=True,
                            )
                        at = attn_p.tile([128, 256], BF16, tag="attn")
                        nc.scalar.activation(at[:], pss[:], EXP, scale=ESCALE)
                        ats[it] = at
                    kc = it - LOOK
                    if kc < 0:
                        continue
                    at = ats[kc]
                    for blk in range(2):
                        if kc < nkt[blk]:
                            last = kc == nkt[blk] - 1
                            nc.tensor.matmul(
                                pc1[blk][:],
                                at[:, blk * 128 : blk * 128 + 128],
                                vtc[kc // 4][:, kc % 4, 0:512],
                                start=(kc == 0),
                                stop=last,
                            )
                            nc.tensor.matmul(
                                pc2[blk][:],
                                at[:, blk * 128 : blk * 128 + 128],
                                vtc[kc // 4][:, kc % 4, 512:769],
                                start=(kc == 0),
                                stop=last,
                            )
                            if last:
                                # normalize + store this q-block immediately;
                                # the two column halves go through different
                                # engines (DVE / Act) and DMA queues so the
                                # tail is latency-minimal.
                                rinv = small_p.tile([128, 1], F32, tag="rinv")
                                nc.vector.reciprocal(rinv[:], pc2[blk][:, 256:257])
                                r = (2 * p + blk) * 128
                                osb1 = out_p.tile([128, 512], F32, tag="osb1")
                                nc.vector.tensor_mul(
                                    osb1[:],
                                    pc1[blk][:],
                                    rinv[:].to_broadcast((128, 512)),
                                )
                                dma_out(out_d[r : r + 128, 0:512], osb1[:])
                                osb2 = out_p.tile([128, 256], F32, tag="osb2")
                                nc.scalar.mul(osb2[:], pc2[blk][:, 0:256], rinv[:])
                                dma_out(out_d[r : r + 128, 512:768], osb2[:])

    nc.compile()
    return nc


def _make_strip(h):
    """[128, 1024] 0/1 mask; block i (256 wide) is added (via -1e30) to the
    S^T psum at diagonal offset i = kt - 4p. Layout [key-row, query-col]."""
    tri = (np.arange(128)[:, None] > np.arange(128)[None, :]).astype(np.float32)
    ones = np.ones((128, 128), np.float32)
    zeros = np.zeros((128, 128), np.float32)
    if h == 0:
        blocks = [(tri, zeros), (ones, zeros), (zeros, tri), (zeros, ones)]
    else:
        blocks = [(zeros, zeros), (tri, zeros), (zeros, zeros), (zeros, tri)]
    return np.concatenate([np.concatenate(b, axis=1) for b in blocks], axis=1)


def _hi_lo(a):
    hi = a.astype(NP_FP8)
    lo = (a - hi.astype(np.float32)).astype(NP_FP8)
    return hi, lo


def kernel(x, Wq, Wk, Wv):
    if "nc" not in _CACHE:
        _CACHE["nc"] = _build()
    nc = _CACHE["nc"]

    x = np.ascontiguousarray(x, dtype=np.float32)
    wq8, wql8 = _hi_lo(WS * np.asarray(Wq, dtype=np.float32).T)
    wk8, wkl8 = _hi_lo(WS * np.asarray(Wk, dtype=np.float32).T)
    wvh8, wvl8 = _hi_lo(WS * np.asarray(Wv, dtype=np.float32).T)
    negd = (NEG * np.eye(128, dtype=np.float32)).astype(NP_BF16)

    in_maps = []
    for c in range(8):
        b, h = c // 2, c % 2
        xbt = np.ascontiguousarray(x[b].T)  # [768, 2048]
        xt8, xlo8 = _hi_lo(xbt)
        # own query columns: pairs p -> global tiles (4p+h, 4p+2+h)
        cols = []
        for p in range(4):
            for g in (4 * p + h, 4 * p + 2 + h):
                cols.append(xbt[:, g * 128 : (g + 1) * 128])
        xqf = np.ascontiguousarray(np.concatenate(cols, axis=1))
        xqt8, xqlo8 = _hi_lo(xqf)
        in_maps.append(
            {
                "xt": xt8,
                "xlo": xlo8,
                "xqt": xqt8,
                "xqlo": xqlo8,
                "wq": wq8,
                "wql": wql8,
                "wk": wk8,
                "wkl": wkl8,
                "wvh": wvh8,
                "wvl": wvl8,
                "strip": _make_strip(h).astype(NP_BF16),
                "negd": negd,
            }
        )

    res = run_bass_kernel_spmd(
        nc,
        in_maps,
        list(range(8)),
        trace=bool(int(os.environ.get("KERNEL_TRACE", "0"))),
    )
    _CACHE["last_results"] = res

    out = np.empty((BATCH, SEQ, D), np.float32)
    for c in range(8):
        b, h = c // 2, c % 2
        o = res.results[c]["out"]
        for p in range(4):
            for blk, g in enumerate((4 * p + h, 4 * p + 2 + h)):
                out[b, g * 128 : (g + 1) * 128] = o[
                    (2 * p + blk) * 128 : (2 * p + blk + 1) * 128
                ]
    return out


# revision 11
# speedup vs baseline: 1.6279x; 1.0339x over previous
"""Causal single-head attention on 8 TRN2 NeuronCores — fp8/bf16 edition.

Problem: x [4, 2048, 768] f32; Wq/Wk/Wv [768, 768] f32 (torch Linear layout).
  q/k/v = x @ W.T ; scores = q k^T causal-masked; attn = softmax(scores/sqrt(768));
  out = attn @ v.

Sharding: core c -> batch b = c//2, half h = c%2. Core h owns global q-tiles
{2lt+h}, grouped into 4 PAIRS: pair p = global tiles (4p+h, 4p+2+h). The
uniform SPMD program processes key-tiles 0..4p+3 for pair p on every core;
which entries are causally masked is pure per-core DATA (the strip input).

Precision strategy (tolerance 2e-2; fp8 DoubleRow matmuls are 4x f32r rate,
bf16 is 2x, in the grading cost model):
  - All weights are pre-scaled by 32 on the host so that both fp8(32W) and
    the fp8 residual fp8(32W - fp8(32W)) sit well above e4m3's minimum
    subnormal (2^-9) — unscaled, |W|<=0.036 makes the residual term flush
    to zero. Projections run as 3-term fp8 DoubleRow hi/lo splits
    (x_hi@W_hi + x_hi@W_lo + x_lo@W_hi), giving ~bf16 accuracy at 75% of
    bf16 PE cost. The x32 scaling cancels: q,k stay scaled (32q, 32k; the
    1024x on scores folds into the exp scale constant), v is unscaled by
    1/32 during psum evacuation (a scaled copy, same cost).
  - QK^T scores: fp8 DoubleRow on fp8-cast 32q/32k (|32q| <= ~130 < 240).
    The only score noise is the fp8 cast; softmax normalization cancels
    common-mode and peaked rows are insensitive. Measured 1.35e-2.
  - attn@V context: bf16 (early causal rows copy v rows verbatim), with a
    ones-column appended to V so the softmax denominator falls out of the
    same matmul (exact normalization even after quantization).

Scores are computed TRANSPOSED (S^T = K Q^T with d on the contraction
partitions): the exp result in [key, query] layout feeds the context matmul
directly as the stationary operand — no PE transposes at all. Causal masking
is done ON THE PE: the last accumulation step of a diagonal tile's psum group
is matmul(diag(-1e30), strip01), adding -1e30 wherever strip==1. Two key
tiles share each [128,512] scores psum so one Activation exp serves both.

Schedule: phase order K -> V -> Q -> attention. K runs three passes over six
open psum groups so its first pass only waits on wk + x_hi chunk0; the
DMA-heavy/PE-light Q inputs stream in the shadow of K/V. DMA count is kept
low (descriptor prep is the bottleneck: one shared HWDGE device at ~625ns/DMA
for SP/Act/DVE queues, Pool software-DGE at ~1038ns/DMA) with a 2:1
sync:gpsimd split. The attention loop is software-pipelined (scores/exp run
4 key-tiles ahead of the context accumulation).
"""

import os
import sys
from contextlib import ExitStack

import numpy as np

for _p in ("/opt/trn_rl_repo", "/root/.axon_site/_ro/trn_rl_repo"):
    if os.path.isdir(_p) and _p not in sys.path:
        sys.path.append(_p)

import ml_dtypes  # noqa: E402

import concourse.mybir as mybir  # noqa: E402
import concourse.tile as tile  # noqa: E402
from concourse import bacc  # noqa: E402
from concourse.bass_utils import run_bass_kernel_spmd  # noqa: E402

F32 = mybir.dt.float32
BF16 = mybir.dt.bfloat16
FP8 = mybir.dt.float8e4
NP_FP8 = ml_dtypes.float8_e4m3
NP_BF16 = ml_dtypes.bfloat16
DR = mybir.MatmulPerfMode.DoubleRow
EXP = mybir.ActivationFunctionType.Exp

BATCH = 4
SEQ = 2048
D = 768
DK = D // 128  # 6 contraction k-tiles; 3 DoubleRow pairs
NQ = 1024  # query rows per core
WS = 32.0  # host-side weight pre-scale
ESCALE = float(1.0 / (np.sqrt(np.float32(D)) * WS * WS))
IVS = float(1.0 / WS)
NEG = -1e30

_CACHE = {}


def _build():
    nc = bacc.Bacc("TRN2", target_bir_lowering=False, debug=False, num_devices=8)
    xt_d = nc.declare_dram_parameter("xt", [D, SEQ], FP8, isOutput=False)
    xlo_d = nc.declare_dram_parameter("xlo", [D, SEQ], FP8, isOutput=False)
    xqt_d = nc.declare_dram_parameter("xqt", [D, NQ], FP8, isOutput=False)
    xqlo_d = nc.declare_dram_parameter("xqlo", [D, NQ], FP8, isOutput=False)
    wq_d = nc.declare_dram_parameter("wq", [D, D], FP8, isOutput=False)
    wql_d = nc.declare_dram_parameter("wql", [D, D], FP8, isOutput=False)
    wk_d = nc.declare_dram_parameter("wk", [D, D], FP8, isOutput=False)
    wkl_d = nc.declare_dram_parameter("wkl", [D, D], FP8, isOutput=False)
    wvh_d = nc.declare_dram_parameter("wvh", [D, D], FP8, isOutput=False)
    wvl_d = nc.declare_dram_parameter("wvl", [D, D], FP8, isOutput=False)
    strip_d = nc.declare_dram_parameter("strip", [128, 1024], BF16, isOutput=False)
    negd_d = nc.declare_dram_parameter("negd", [128, 128], BF16, isOutput=False)
    out_d = nc.declare_dram_parameter("out", [NQ, D], F32, isOutput=True)

    # 2:1 split between the SP HWDGE queue and the Pool SWDGE queue: one
    # shared HWDGE device serves SP/Act/DVE at ~625ns/DMA prep; Pool preps in
    # software (~1038ns) but on its own engine, in parallel. Emission order is
    # the prefetch schedule (DMA transfers serialize on the DMA engines).
    _dma_i = [0]

    def dma_in(dst, src):
        eng = (nc.sync, nc.gpsimd, nc.sync)[_dma_i[0] % 3]
        eng.dma_start(dst, src)
        _dma_i[0] += 1

    # Psum evacuations alternate DVE / Act; V variant fuses the 1/32 unscale.
    _evac_i = [0]

    def evac(dst, src, scale=None):
        if _evac_i[0] % 2 == 0:
            if scale is None:
                nc.vector.tensor_copy(dst, src)
            else:
                nc.vector.tensor_scalar_mul(dst, src, scale)
        else:
            if scale is None:
                nc.scalar.copy(dst, src)
            else:
                nc.scalar.mul(dst, src, scale)
        _evac_i[0] += 1

    def rearr(dram_slice):
        return dram_slice.rearrange("(ko p) s -> p ko s", p=128)

    def xsl(chunk, j, cs):
        """ko-pair j view of an x chunk: list of 3 [128,2,512] tiles, or one
        monolithic [128,6,512] tile."""
        if isinstance(chunk, list):
            return chunk[j][:, :, cs]
        return chunk[:, 2 * j : 2 * j + 2, cs]

    FULL = slice(0, 512)

    with tile.TileContext(nc) as tc, ExitStack() as ctx:
        persist = ctx.enter_context(tc.tile_pool(name="persist", bufs=1))

        qth = [persist.tile([128, DK, 512], FP8, name=f"qth{i}") for i in range(2)]
        kt = persist.tile([128, DK, SEQ], FP8)  # K^T resident (fp8 cast, 32-scaled)
        vtc = [persist.tile([128, 4, 769], BF16, name=f"vtc{i}") for i in range(4)]
        strip = persist.tile([128, 1024], BF16)
        negd = persist.tile([128, 128], BF16)

        with ExitStack() as pw:
            xp = pw.enter_context(tc.tile_pool(name="xp", bufs=1))
            wkp = pw.enter_context(tc.tile_pool(name="wkp", bufs=1))
            wvp = pw.enter_context(tc.tile_pool(name="wvp", bufs=1))
            wqp = pw.enter_context(tc.tile_pool(name="wqp", bufs=1))
            xqp = pw.enter_context(tc.tile_pool(name="xqp", bufs=1))

            # ---------------- Phase K: K^T projection ----------------
            # Three passes over six open psum groups: pass 1 (wk * x_hi) only
            # needs wk + chunk0_hi, so the PE starts after ~1MB of DMA and the
            # pass-2/3 operands stream in behind it.
            with ExitStack() as pK:
                psk = pK.enter_context(tc.tile_pool(name="psk", bufs=6, space="PSUM"))
                wk = [wkp.tile([128, 2, D], FP8, name=f"wk{j}") for j in range(3)]
                wkl = [wkp.tile([128, 2, D], FP8, name=f"wkl{j}") for j in range(3)]
                xt8c = [
                    [xp.tile([128, 2, 512], FP8, name=f"xt8c0_{j}") for j in range(3)]
                ]
                xlo8c = [
                    [xp.tile([128, 2, 512], FP8, name=f"xlo8c0_{j}") for j in range(3)]
                ]
                for sc in range(1, 4):
                    xt8c.append(xp.tile([128, DK, 512], FP8, name=f"xt8c{sc}"))
                    xlo8c.append(xp.tile([128, DK, 512], FP8, name=f"xlo8c{sc}"))
                # pass-1 operands first, j-interleaved
                for j in range(3):
                    dma_in(wk[j][:], rearr(wk_d[j * 256 : (j + 1) * 256, :]))
                    dma_in(xt8c[0][j][:], rearr(xt_d[j * 256 : (j + 1) * 256, 0:512]))
                for j in range(3):
                    dma_in(wkl[j][:], rearr(wkl_d[j * 256 : (j + 1) * 256, :]))
                for j in range(3):
                    dma_in(xlo8c[0][j][:], rearr(xlo_d[j * 256 : (j + 1) * 256, 0:512]))

                for sc in range(4):
                    if sc >= 1:
                        dma_in(xt8c[sc][:], rearr(xt_d[:, sc * 512 : sc * 512 + 512]))
                        dma_in(xlo8c[sc][:], rearr(xlo_d[:, sc * 512 : sc * 512 + 512]))
                    if sc == 2:
                        wvh = wvp.tile([128, DK, D], FP8, name="wvh")
                        wvl = wvp.tile([128, DK, D], FP8, name="wvl")
                        dma_in(wvh[:], rearr(wvh_d[:]))
                        dma_in(wvl[:], rearr(wvl_d[:]))
                    pks = [psk.tile([128, 512], F32, tag="psk", name=f"pks{_oo}") for _oo in range(DK)]
                    for pi, (wop, xop) in enumerate(
                        ((wk, xt8c[sc]), (wkl, xt8c[sc]), (wk, xlo8c[sc]))
                    ):
                        for oo in range(DK):
                            for j in range(3):
                                nc.tensor.matmul(
                                    pks[oo][:],
                                    wop[j][:, :, oo * 128 : oo * 128 + 128],
                                    xsl(xop, j, FULL),
                                    perf_mode=DR,
                                    start=(pi == 0 and j == 0),
                                    stop=(pi == 2 and j == 2),
                                )
                    for oo in range(DK):
                        evac(kt[:, oo, sc * 512 : sc * 512 + 512], pks[oo][:])

            # ---------------- Phase V: V projection ----------------
            with ExitStack() as pV:
                psv = pV.enter_context(tc.tile_pool(name="psv", bufs=3, space="PSUM"))
                xq8 = xqp.tile([128, DK, NQ], FP8, name="xq8")
                xql8 = xqp.tile([128, DK, NQ], FP8, name="xql8")
                wq = wqp.tile([128, DK, D], FP8, name="wq")
                wql = wqp.tile([128, DK, D], FP8, name="wql")
                dma_in(wq[:], rearr(wq_d[:]))
                dma_in(xq8[:], rearr(xqt_d[:]))
                dma_in(wql[:], rearr(wql_d[:]))
                dma_in(xql8[:], rearr(xqlo_d[:]))
                dma_in(strip[:], strip_d[:])
                dma_in(negd[:], negd_d[:])
                for sc in range(4):
                    nc.vector.memset(vtc[sc][:, :, 768:769], 1.0)
                    for st in range(4):
                        stc = slice(st * 128, st * 128 + 128)
                        for oc in range(2):
                            pv = psv.tile([128, 384], F32, tag="psv")
                            n = 0
                            for xop, wop in (
                                (xt8c[sc], wvh),
                                (xt8c[sc], wvl),
                                (xlo8c[sc], wvh),
                            ):
                                for j in range(3):
                                    nc.tensor.matmul(
                                        pv[:],
                                        xsl(xop, j, stc),
                                        wop[:, 2 * j : 2 * j + 2, oc * 384 : oc * 384 + 384],
                                        perf_mode=DR,
                                        start=(n == 0),
                                        stop=(n == 8),
                                    )
                                    n += 1
                            evac(vtc[sc][:, st, oc * 384 : oc * 384 + 384], pv[:], scale=IVS)

            # ---------------- Phase Q: Q^T projection ----------------
            with ExitStack() as pQ:
                psq = pQ.enter_context(tc.tile_pool(name="psq", bufs=3, space="PSUM"))
                for sc in range(2):
                    scc = slice(sc * 512, sc * 512 + 512)
                    for oo in range(DK):
                        pq = psq.tile([128, 512], F32, tag="psq")
                        n = 0
                        for wop, xop in ((wq, xq8), (wql, xq8), (wq, xql8)):
                            for j in range(3):
                                nc.tensor.matmul(
                                    pq[:],
                                    wop[:, 2 * j : 2 * j + 2, oo * 128 : oo * 128 + 128],
                                    xop[:, 2 * j : 2 * j + 2, scc],
                                    perf_mode=DR,
                                    start=(n == 0),
                                    stop=(n == 8),
                                )
                                n += 1
                        evac(qth[sc][:, oo, :], pq[:])

        # ---------------- Phase A: attention (S^T scheme) ----------------
        with ExitStack() as pA:
            pss_p = pA.enter_context(tc.tile_pool(name="pss", bufs=2, space="PSUM"))
            pc1a_p = pA.enter_context(tc.tile_pool(name="pc1a", bufs=2, space="PSUM"))
            pc2a_p = pA.enter_context(tc.tile_pool(name="pc2a", bufs=1, space="PSUM"))
            pc1b_p = pA.enter_context(tc.tile_pool(name="pc1b", bufs=2, space="PSUM"))
            pc2b_p = pA.enter_context(tc.tile_pool(name="pc2b", bufs=1, space="PSUM"))
            attn_p = pA.enter_context(tc.tile_pool(name="attn", bufs=4))
            out_p = pA.enter_context(tc.tile_pool(name="outp", bufs=2))
            small_p = pA.enter_context(tc.tile_pool(name="small", bufs=2))

            LOOK = 2  # pipeline depth in steps (1 step = 2 key-tiles)

            def emit_scores(p, s):
                """Scores+exp for key-tiles (2s, 2s+1) of pair p; one shared
                [128,512] psum, one exp."""
                pss = pss_p.tile([128, 512], F32, tag="pss")
                for half in range(2):
                    kt_i = 2 * s + half
                    di = kt_i - 4 * p
                    hc = slice(half * 256, half * 256 + 256)
                    for j in range(3):
                        nc.tensor.matmul(
                            pss[:, hc],
                            kt[:, 2 * j : 2 * j + 2, kt_i * 128 : kt_i * 128 + 128],
                            qth[p // 2][
                                :, 2 * j : 2 * j + 2,
                                (p % 2) * 256 : (p % 2) * 256 + 256,
                            ],
                            perf_mode=DR,
                            start=(j == 0),
                            stop=(j == 2 and di < 0),
                        )
                    if di >= 0:
                        # causal mask on the PE: psum += -1e30 * strip01
                        nc.tensor.matmul(
                            pss[:, hc],
                            negd[:],
                            strip[:, di * 256 : di * 256 + 256],
                            start=False,
                            stop=True,
                        )
                at = attn_p.tile([128, 512], BF16, tag="attn")
                nc.scalar.activation(at[:], pss[:], EXP, scale=ESCALE)
                return at

            def emit_ctx(p, s, at, pc1, pc2, nkt):
                for half in range(2):
                    kt_i = 2 * s + half
                    off = half * 256
                    for blk in range(2):
                        if kt_i >= nkt[blk]:
                            continue
                        last = kt_i == nkt[blk] - 1
                        lhsT = at[:, off + blk * 128 : off + blk * 128 + 128]
                        nc.tensor.matmul(
                            pc1[blk][:],
                            lhsT,
                            vtc[kt_i // 4][:, kt_i % 4, 0:512],
                            start=(kt_i == 0),
                            stop=last,
                        )
                        nc.tensor.matmul(
                            pc2[blk][:],
                            lhsT,
                            vtc[kt_i // 4][:, kt_i % 4, 512:769],
                            start=(kt_i == 0),
                            stop=last,
                        )
                        if last:
                            # normalize + store this q-block immediately; the
                            # two column halves use different engines (DVE /
                            # Act); out DMAs ride the HWDGE (sync) queue.
                            rinv = small_p.tile([128, 1], F32, tag="rinv")
                            nc.vector.reciprocal(rinv[:], pc2[blk][:, 256:257])
                            r = (2 * p + blk) * 128
                            osb1 = out_p.tile([128, 512], F32, tag="osb1")
                            nc.vector.tensor_mul(
                                osb1[:], pc1[blk][:], rinv[:].to_broadcast((128, 512))
                            )
                            nc.sync.dma_start(out_d[r : r + 128, 0:512], osb1[:])
                            osb2 = out_p.tile([128, 256], F32, tag="osb2")
                            nc.scalar.mul(osb2[:], pc2[blk][:, 0:256], rinv[:])
                            nc.sync.dma_start(out_d[r : r + 128, 512:768], osb2[:])

            for p in range(4):
                pc1 = [
                    pc1a_p.tile([128, 512], F32, name="pc1a"),
                    pc1b_p.tile([128, 512], F32, name="pc1b"),
                ]
                pc2 = [
                    pc2a_p.tile([128, 257], F32, name="pc2a"),
                    pc2b_p.tile([128, 257], F32, name="pc2b"),
                ]
                nkt = [4 * p + 2, 4 * p + 4]  # ctx key-tile count per q-block
                S = 2 * p + 2  # steps (2 key-tiles each)
                ats = {}
                for it in range(S + LOOK):
                    if it < S:
                        ats[it] = emit_scores(p, it)
                    sc = it - LOOK
                    if sc >= 0:
                        emit_ctx(p, sc, ats.pop(sc), pc1, pc2, nkt)

    nc.compile()
    return nc


def _make_strip(h):
    """[128, 1024] 0/1 mask; block i (256 wide) is added (via -1e30) to the
    S^T psum at diagonal offset i = kt - 4p. Layout [key-row, query-col]."""
    tri = (np.arange(128)[:, None] > np.arange(128)[None, :]).astype(np.float32)
    ones = np.ones((128, 128), np.float32)
    zeros = np.zeros((128, 128), np.float32)
    if h == 0:
        blocks = [(tri, zeros), (ones, zeros), (zeros, tri), (zeros, ones)]
    else:
        blocks = [(zeros, zeros), (tri, zeros), (zeros, zeros), (zeros, tri)]
    return np.concatenate([np.concatenate(b, axis=1) for b in blocks], axis=1)


def _hi_lo(a):
    hi = a.astype(NP_FP8)
    lo = (a - hi.astype(np.float32)).astype(NP_FP8)
    return hi, lo


def kernel(x, Wq, Wk, Wv):
    if "nc" not in _CACHE:
        _CACHE["nc"] = _build()
    nc = _CACHE["nc"]

    x = np.ascontiguousarray(x, dtype=np.float32)
    wq8, wql8 = _hi_lo(WS * np.asarray(Wq, dtype=np.float32).T)
    wk8, wkl8 = _hi_lo(WS * np.asarray(Wk, dtype=np.float32).T)
    wvh8, wvl8 = _hi_lo(WS * np.asarray(Wv, dtype=np.float32).T)
    negd = (NEG * np.eye(128, dtype=np.float32)).astype(NP_BF16)

    in_maps = []
    for c in range(8):
        b, h = c // 2, c % 2
        xbt = np.ascontiguousarray(x[b].T)  # [768, 2048]
        xt8, xlo8 = _hi_lo(xbt)
        # own query columns: pairs p -> global tiles (4p+h, 4p+2+h)
        cols = []
        for p in range(4):
            for g in (4 * p + h, 4 * p + 2 + h):
                cols.append(xbt[:, g * 128 : (g + 1) * 128])
        xqf = np.ascontiguousarray(np.concatenate(cols, axis=1))
        xqt8, xqlo8 = _hi_lo(xqf)
        in_maps.append(
            {
                "xt": xt8,
                "xlo": xlo8,
                "xqt": xqt8,
                "xqlo": xqlo8,
                "wq": wq8,
                "wql": wql8,
                "wk": wk8,
                "wkl": wkl8,
                "wvh": wvh8,
                "wvl": wvl8,
                "strip": _make_strip(h).astype(NP_BF16),
                "negd": negd,
            }
        )

    res = run_bass_kernel_spmd(
        nc,
        in_maps,
        list(range(8)),
        trace=bool(int(os.environ.get("KERNEL_TRACE", "0"))),
    )
    _CACHE["last_results"] = res

    out = np.empty((BATCH, SEQ, D), np.float32)
    for c in range(8):
        b, h = c // 2, c % 2
        o = res.results[c]["out"]
        for p in range(4):
            for blk, g in enumerate((4 * p + h, 4 * p + 2 + h)):
                out[b, g * 128 : (g + 1) * 128] = o[
                    (2 * p + blk) * 128 : (2 * p + blk + 1) * 128
                ]
    return out


# revision 12
# speedup vs baseline: 1.8091x; 1.1113x over previous
"""Causal single-head attention on 8 TRN2 NeuronCores — fp8/bf16 edition.

Problem: x [4, 2048, 768] f32; Wq/Wk/Wv [768, 768] f32 (torch Linear layout).
  q/k/v = x @ W.T ; scores = q k^T causal-masked; attn = softmax(scores/sqrt(768));
  out = attn @ v.

Sharding: core c -> batch b = c//2, half h = c%2. Core h owns global q-tiles
{2lt+h}, grouped into 4 PAIRS: pair p = global tiles (4p+h, 4p+2+h). The
uniform SPMD program processes key-tiles 0..4p+3 for pair p on every core;
which entries are causally masked is pure per-core DATA (the strip input).

Precision strategy (tolerance 2e-2; fp8 DoubleRow matmuls are 4x f32r rate,
bf16 is 2x, in the grading cost model):
  - All weights are pre-scaled by 32 on the host so that both fp8(32W) and
    the fp8 residual fp8(32W - fp8(32W)) sit well above e4m3's minimum
    subnormal (2^-9) — unscaled, |W|<=0.036 makes the residual term flush
    to zero. Projections run as 3-term fp8 DoubleRow hi/lo splits
    (x_hi@W_hi + x_hi@W_lo + x_lo@W_hi), giving ~bf16 accuracy at 75% of
    bf16 PE cost. The x32 scaling cancels: q,k stay scaled (32q, 32k; the
    1024x on scores folds into the exp scale constant), v is unscaled by
    1/32 during psum evacuation (a scaled copy, same cost).
  - QK^T scores: fp8 DoubleRow on fp8-cast 32q/32k (|32q| <= ~130 < 240).
    The only score noise is the fp8 cast; softmax normalization cancels
    common-mode and peaked rows are insensitive. Measured 1.35e-2.
  - attn@V context: bf16 (early causal rows copy v rows verbatim), with a
    ones-column appended to V so the softmax denominator falls out of the
    same matmul (exact normalization even after quantization).

Scores are computed TRANSPOSED (S^T = K Q^T with d on the contraction
partitions): the exp result in [key, query] layout feeds the context matmul
directly as the stationary operand — no PE transposes at all. Causal masking
is done ON THE PE: the last accumulation step of a diagonal tile's psum group
is matmul(diag(-1e30), strip01), adding -1e30 wherever strip==1. Two key
tiles share each [128,512] scores psum so one Activation exp serves both.

Schedule: phase order K -> V -> Q -> attention. K runs three passes over six
open psum groups so its first pass only waits on wk + x_hi chunk0; the
DMA-heavy/PE-light Q inputs stream in the shadow of K/V. DMA count is kept
low (descriptor prep is the bottleneck: one shared HWDGE device at ~625ns/DMA
for SP/Act/DVE queues, Pool software-DGE at ~1038ns/DMA) with a 2:1
sync:gpsimd split. The attention loop is software-pipelined (scores/exp run
4 key-tiles ahead of the context accumulation).
"""

import os
import sys
from contextlib import ExitStack

import numpy as np

for _p in ("/opt/trn_rl_repo", "/root/.axon_site/_ro/trn_rl_repo"):
    if os.path.isdir(_p) and _p not in sys.path:
        sys.path.append(_p)

import ml_dtypes  # noqa: E402

import concourse.mybir as mybir  # noqa: E402
import concourse.tile as tile  # noqa: E402
from concourse import bacc  # noqa: E402
from concourse.bass_utils import run_bass_kernel_spmd  # noqa: E402

F32 = mybir.dt.float32
BF16 = mybir.dt.bfloat16
FP8 = mybir.dt.float8e4
NP_FP8 = ml_dtypes.float8_e4m3
NP_BF16 = ml_dtypes.bfloat16
DR = mybir.MatmulPerfMode.DoubleRow
EXP = mybir.ActivationFunctionType.Exp

BATCH = 4
SEQ = 2048
D = 768
DK = D // 128  # 6 contraction k-tiles; 3 DoubleRow pairs
NQ = 1024  # query rows per core
WS = 32.0  # host-side weight pre-scale
ESCALE = float(1.0 / (np.sqrt(np.float32(D)) * WS * WS))
IVS = float(1.0 / WS)
NEG = -1e30

_CACHE = {}


def _build():
    nc = bacc.Bacc("TRN2", target_bir_lowering=False, debug=False, num_devices=8)
    xt_d = nc.declare_dram_parameter("xt", [D, SEQ], FP8, isOutput=False)
    xlo_d = nc.declare_dram_parameter("xlo", [D, SEQ], FP8, isOutput=False)
    xqt_d = nc.declare_dram_parameter("xqt", [D, NQ], FP8, isOutput=False)
    wq_d = nc.declare_dram_parameter("wq", [D, D], FP8, isOutput=False)
    wql_d = nc.declare_dram_parameter("wql", [D, D], FP8, isOutput=False)
    wk_d = nc.declare_dram_parameter("wk", [D, D], FP8, isOutput=False)
    wkl_d = nc.declare_dram_parameter("wkl", [D, D], FP8, isOutput=False)
    wvh_d = nc.declare_dram_parameter("wvh", [D, D], FP8, isOutput=False)
    wvl_d = nc.declare_dram_parameter("wvl", [D, D], FP8, isOutput=False)
    strip_d = nc.declare_dram_parameter("strip", [128, 512], BF16, isOutput=False)
    negd_d = nc.declare_dram_parameter("negd", [128, 128], BF16, isOutput=False)
    out_d = nc.declare_dram_parameter("out", [NQ, D], F32, isOutput=True)

    # 2:1 split between the SP HWDGE queue and the Pool SWDGE queue: one
    # shared HWDGE device serves SP/Act/DVE at ~625ns/DMA prep; Pool preps in
    # software (~1038ns) but on its own engine, in parallel. Emission order is
    # the prefetch schedule (DMA transfers serialize on the DMA engines).
    _dma_i = [0]

    def dma_in(dst, src):
        eng = (nc.sync, nc.gpsimd, nc.sync)[_dma_i[0] % 3]
        eng.dma_start(dst, src)
        _dma_i[0] += 1

    # Psum evacuations alternate DVE / Act; V variant fuses the 1/32 unscale.
    _evac_i = [0]

    def evac(dst, src, scale=None):
        if _evac_i[0] % 2 == 0:
            if scale is None:
                nc.vector.tensor_copy(dst, src)
            else:
                nc.vector.tensor_scalar_mul(dst, src, scale)
        else:
            if scale is None:
                nc.scalar.copy(dst, src)
            else:
                nc.scalar.mul(dst, src, scale)
        _evac_i[0] += 1

    def rearr(dram_slice):
        return dram_slice.rearrange("(ko p) s -> p ko s", p=128)

    def xsl(chunk, j, cs):
        """ko-pair j view of an x chunk: list of 3 [128,2,512] tiles, or one
        monolithic [128,6,512] tile."""
        if isinstance(chunk, list):
            return chunk[j][:, :, cs]
        return chunk[:, 2 * j : 2 * j + 2, cs]

    FULL = slice(0, 512)

    with tile.TileContext(nc) as tc, ExitStack() as ctx:
        persist = ctx.enter_context(tc.tile_pool(name="persist", bufs=1))

        qth = [persist.tile([128, DK, 512], FP8, name=f"qth{i}") for i in range(2)]
        kt = persist.tile([128, DK, SEQ], FP8)  # K^T resident (fp8 cast, 32-scaled)
        vtc = [persist.tile([128, 4, 769], BF16, name=f"vtc{i}") for i in range(4)]
        strip = persist.tile([128, 512], BF16)
        negd = persist.tile([128, 128], BF16)

        with ExitStack() as pw:
            xp = pw.enter_context(tc.tile_pool(name="xp", bufs=1))
            wkp = pw.enter_context(tc.tile_pool(name="wkp", bufs=1))
            wvp = pw.enter_context(tc.tile_pool(name="wvp", bufs=1))
            wqp = pw.enter_context(tc.tile_pool(name="wqp", bufs=1))
            xqp = pw.enter_context(tc.tile_pool(name="xqp", bufs=1))

            # ---------------- Phase K: K^T projection ----------------
            # Three passes over six open psum groups: pass 1 (wk * x_hi) only
            # needs wk + chunk0_hi, so the PE starts after ~1MB of DMA and the
            # pass-2/3 operands stream in behind it.
            with ExitStack() as pK:
                psk = pK.enter_context(tc.tile_pool(name="psk", bufs=6, space="PSUM"))
                wk = [wkp.tile([128, 2, D], FP8, name=f"wk{j}") for j in range(3)]
                wkl = [wkp.tile([128, 2, D], FP8, name=f"wkl{j}") for j in range(3)]
                xt8c = [
                    [xp.tile([128, 2, 512], FP8, name=f"xt8c0_{j}") for j in range(3)]
                ]
                xlo8c = [
                    [xp.tile([128, 2, 512], FP8, name=f"xlo8c0_{j}") for j in range(3)]
                ]
                for sc in range(1, 4):
                    xt8c.append(xp.tile([128, DK, 512], FP8, name=f"xt8c{sc}"))
                    xlo8c.append(xp.tile([128, DK, 512], FP8, name=f"xlo8c{sc}"))
                # pass-1 operands first, j-interleaved
                for j in range(3):
                    dma_in(wk[j][:], rearr(wk_d[j * 256 : (j + 1) * 256, :]))
                    dma_in(xt8c[0][j][:], rearr(xt_d[j * 256 : (j + 1) * 256, 0:512]))
                for j in range(3):
                    dma_in(wkl[j][:], rearr(wkl_d[j * 256 : (j + 1) * 256, :]))

                for sc in range(4):
                    if sc >= 1:
                        dma_in(xt8c[sc][:], rearr(xt_d[:, sc * 512 : sc * 512 + 512]))
                        dma_in(xlo8c[sc][:], rearr(xlo_d[:, sc * 512 : sc * 512 + 512]))
                    if sc == 2:
                        wvh = wvp.tile([128, DK, D], FP8, name="wvh")
                        wvl = wvp.tile([128, DK, D], FP8, name="wvl")
                        dma_in(wvh[:], rearr(wvh_d[:]))
                        dma_in(wvl[:], rearr(wvl_d[:]))
                    if sc == 0:
                        for j in range(3):
                            dma_in(xlo8c[0][j][:], rearr(xlo_d[j * 256 : (j + 1) * 256, 0:512]))
                    pks = [psk.tile([128, 512], F32, tag="psk", name=f"pks{_oo}") for _oo in range(DK)]
                    for pi, (wop, xop) in enumerate(((wk, xt8c[sc]), (wkl, xt8c[sc]))):
                        for oo in range(DK):
                            for j in range(3):
                                nc.tensor.matmul(
                                    pks[oo][:],
                                    wop[j][:, :, oo * 128 : oo * 128 + 128],
                                    xsl(xop, j, FULL),
                                    perf_mode=DR,
                                    start=(pi == 0 and j == 0),
                                    stop=(pi == 1 and j == 2),
                                )
                    for oo in range(DK):
                        evac(kt[:, oo, sc * 512 : sc * 512 + 512], pks[oo][:])

            # ---------------- Phase V: V projection ----------------
            with ExitStack() as pV:
                psv = pV.enter_context(tc.tile_pool(name="psv", bufs=3, space="PSUM"))
                xq8 = xqp.tile([128, DK, NQ], FP8, name="xq8")
                wq = wqp.tile([128, DK, D], FP8, name="wq")
                wql = wqp.tile([128, DK, D], FP8, name="wql")
                dma_in(wq[:], rearr(wq_d[:]))
                dma_in(xq8[:], rearr(xqt_d[:]))
                dma_in(wql[:], rearr(wql_d[:]))
                dma_in(strip[:], strip_d[:])
                dma_in(negd[:], negd_d[:])
                for sc in range(4):
                    nc.vector.memset(vtc[sc][:, :, 768:769], 1.0)
                    for st in range(4):
                        stc = slice(st * 128, st * 128 + 128)
                        for oc in range(2):
                            pv = psv.tile([128, 384], F32, tag="psv")
                            n = 0
                            for xop, wop in (
                                (xt8c[sc], wvh),
                                (xt8c[sc], wvl),
                                (xlo8c[sc], wvh),
                            ):
                                for j in range(3):
                                    nc.tensor.matmul(
                                        pv[:],
                                        xsl(xop, j, stc),
                                        wop[:, 2 * j : 2 * j + 2, oc * 384 : oc * 384 + 384],
                                        perf_mode=DR,
                                        start=(n == 0),
                                        stop=(n == 8),
                                    )
                                    n += 1
                            evac(vtc[sc][:, st, oc * 384 : oc * 384 + 384], pv[:], scale=IVS)

            # ---------------- Phase Q: Q^T projection ----------------
            with ExitStack() as pQ:
                psq = pQ.enter_context(tc.tile_pool(name="psq", bufs=3, space="PSUM"))
                for sc in range(2):
                    scc = slice(sc * 512, sc * 512 + 512)
                    for oo in range(DK):
                        pq = psq.tile([128, 512], F32, tag="psq")
                        n = 0
                        for wop in (wq, wql):
                            for j in range(3):
                                nc.tensor.matmul(
                                    pq[:],
                                    wop[:, 2 * j : 2 * j + 2, oo * 128 : oo * 128 + 128],
                                    xq8[:, 2 * j : 2 * j + 2, scc],
                                    perf_mode=DR,
                                    start=(n == 0),
                                    stop=(n == 5),
                                )
                                n += 1
                        evac(qth[sc][:, oo, :], pq[:])

        # ---------------- Phase A: attention (S^T scheme) ----------------
        with ExitStack() as pA:
            pss_p = pA.enter_context(tc.tile_pool(name="pss", bufs=2, space="PSUM"))
            pc1a_p = pA.enter_context(tc.tile_pool(name="pc1a", bufs=2, space="PSUM"))
            pc2a_p = pA.enter_context(tc.tile_pool(name="pc2a", bufs=1, space="PSUM"))
            pc1b_p = pA.enter_context(tc.tile_pool(name="pc1b", bufs=2, space="PSUM"))
            pc2b_p = pA.enter_context(tc.tile_pool(name="pc2b", bufs=1, space="PSUM"))
            attn_p = pA.enter_context(tc.tile_pool(name="attn", bufs=4))
            out_p = pA.enter_context(tc.tile_pool(name="outp", bufs=2))
            small_p = pA.enter_context(tc.tile_pool(name="small", bufs=2))

            LOOK = 2  # pipeline depth in steps (1 step = 2 key-tiles)

            def emit_scores(p, s):
                """Scores+exp for key-tiles (2s, 2s+1) of pair p; one shared
                [128,512] psum, one exp."""
                pss = pss_p.tile([128, 512], F32, tag="pss")
                for half in range(2):
                    kt_i = 2 * s + half
                    di = kt_i - 4 * p
                    hc = slice(half * 256, half * 256 + 256)
                    for j in range(3):
                        nc.tensor.matmul(
                            pss[:, hc],
                            kt[:, 2 * j : 2 * j + 2, kt_i * 128 : kt_i * 128 + 128],
                            qth[p // 2][
                                :, 2 * j : 2 * j + 2,
                                (p % 2) * 256 : (p % 2) * 256 + 256,
                            ],
                            perf_mode=DR,
                            start=(j == 0),
                            stop=(j == 2 and di < 0),
                        )
                    if di >= 0:
                        # causal mask on the PE: psum += -1e30 * strip01.
                        # Only one q-block can need masking at offset di
                        # (block0 for di<2, block1 for di>=2); which CORE
                        # masks is encoded in the strip data.
                        blkpos = 0 if di < 2 else 1
                        nc.tensor.matmul(
                            pss[:, half * 256 + blkpos * 128 : half * 256 + blkpos * 128 + 128],
                            negd[:],
                            strip[:, di * 128 : di * 128 + 128],
                            start=False,
                            stop=True,
                        )
                at = attn_p.tile([128, 512], BF16, tag="attn")
                nc.scalar.activation(at[:], pss[:], EXP, scale=ESCALE)
                return at

            def emit_ctx(p, s, at, pc1, pc2, nkt):
                for half in range(2):
                    kt_i = 2 * s + half
                    off = half * 256
                    for blk in range(2):
                        if kt_i >= nkt[blk]:
                            continue
                        last = kt_i == nkt[blk] - 1
                        lhsT = at[:, off + blk * 128 : off + blk * 128 + 128]
                        nc.tensor.matmul(
                            pc1[blk][:],
                            lhsT,
                            vtc[kt_i // 4][:, kt_i % 4, 0:512],
                            start=(kt_i == 0),
                            stop=last,
                        )
                        nc.tensor.matmul(
                            pc2[blk][:],
                            lhsT,
                            vtc[kt_i // 4][:, kt_i % 4, 512:769],
                            start=(kt_i == 0),
                            stop=last,
                        )
                        if last:
                            # normalize + store this q-block immediately; the
                            # two column halves use different engines (DVE /
                            # Act); out DMAs ride the HWDGE (sync) queue.
                            rinv = small_p.tile([128, 1], F32, tag="rinv")
                            nc.vector.reciprocal(rinv[:], pc2[blk][:, 256:257])
                            r = (2 * p + blk) * 128
                            osb = out_p.tile([128, D], F32, tag="osb")
                            nc.vector.tensor_mul(
                                osb[:, 0:512], pc1[blk][:], rinv[:].to_broadcast((128, 512))
                            )
                            nc.scalar.mul(osb[:, 512:768], pc2[blk][:, 0:256], rinv[:])
                            nc.sync.dma_start(out_d[r : r + 128, :], osb[:])

            for p in range(4):
                pc1 = [
                    pc1a_p.tile([128, 512], F32, name="pc1a"),
                    pc1b_p.tile([128, 512], F32, name="pc1b"),
                ]
                pc2 = [
                    pc2a_p.tile([128, 257], F32, name="pc2a"),
                    pc2b_p.tile([128, 257], F32, name="pc2b"),
                ]
                nkt = [4 * p + 2, 4 * p + 4]  # ctx key-tile count per q-block
                S = 2 * p + 2  # steps (2 key-tiles each)
                ats = {}
                for it in range(S + LOOK):
                    if it < S:
                        ats[it] = emit_scores(p, it)
                    sc = it - LOOK
                    if sc >= 0:
                        emit_ctx(p, sc, ats.pop(sc), pc1, pc2, nkt)

    nc.compile()
    return nc


def _make_strip(h):
    """[128, 512] 0/1 mask; block i (128 wide) is added (via -1e30) to the
    masked q-block at diagonal offset i = kt - 4p. [key-row, query-col]."""
    tri = (np.arange(128)[:, None] > np.arange(128)[None, :]).astype(np.float32)
    ones = np.ones((128, 128), np.float32)
    zeros = np.zeros((128, 128), np.float32)
    blocks = [tri, ones, tri, ones] if h == 0 else [zeros, tri, zeros, tri]
    return np.concatenate(blocks, axis=1)


def _hi_lo(a):
    hi = a.astype(NP_FP8)
    lo = (a - hi.astype(np.float32)).astype(NP_FP8)
    return hi, lo


def kernel(x, Wq, Wk, Wv):
    if "nc" not in _CACHE:
        _CACHE["nc"] = _build()
    nc = _CACHE["nc"]

    x = np.ascontiguousarray(x, dtype=np.float32)
    wq8, wql8 = _hi_lo(WS * np.asarray(Wq, dtype=np.float32).T)
    wk8, wkl8 = _hi_lo(WS * np.asarray(Wk, dtype=np.float32).T)
    wvh8, wvl8 = _hi_lo(WS * np.asarray(Wv, dtype=np.float32).T)
    negd = (NEG * np.eye(128, dtype=np.float32)).astype(NP_BF16)

    in_maps = []
    for c in range(8):
        b, h = c // 2, c % 2
        xbt = np.ascontiguousarray(x[b].T)  # [768, 2048]
        xt8, xlo8 = _hi_lo(xbt)
        # own query columns: pairs p -> global tiles (4p+h, 4p+2+h)
        cols = []
        for p in range(4):
            for g in (4 * p + h, 4 * p + 2 + h):
                cols.append(xbt[:, g * 128 : (g + 1) * 128])
        xqt8 = np.ascontiguousarray(np.concatenate(cols, axis=1)).astype(NP_FP8)
        in_maps.append(
            {
                "xt": xt8,
                "xlo": xlo8,
                "xqt": xqt8,
                "wq": wq8,
                "wql": wql8,
                "wk": wk8,
                "wkl": wkl8,
                "wvh": wvh8,
                "wvl": wvl8,
                "strip": _make_strip(h).astype(NP_BF16),
                "negd": negd,
            }
        )

    res = run_bass_kernel_spmd(
        nc,
        in_maps,
        list(range(8)),
        trace=bool(int(os.environ.get("KERNEL_TRACE", "0"))),
    )
    _CACHE["last_results"] = res

    out = np.empty((BATCH, SEQ, D), np.float32)
    for c in range(8):
        b, h = c // 2, c % 2
        o = res.results[c]["out"]
        for p in range(4):
            for blk, g in enumerate((4 * p + h, 4 * p + 2 + h)):
                out[b, g * 128 : (g + 1) * 128] = o[
                    (2 * p + blk) * 128 : (2 * p + blk + 1) * 128
                ]
    return out


# revision 13
# speedup vs baseline: 1.8103x; 1.0006x over previous
"""Causal single-head attention on 8 TRN2 NeuronCores — fp8/bf16 edition.

Problem: x [4, 2048, 768] f32; Wq/Wk/Wv [768, 768] f32 (torch Linear layout).
  q/k/v = x @ W.T ; scores = q k^T causal-masked; attn = softmax(scores/sqrt(768));
  out = attn @ v.

Sharding: core c -> batch b = c//2, half h = c%2. Core h owns global q-tiles
{2lt+h}, grouped into 4 PAIRS: pair p = global tiles (4p+h, 4p+2+h). The
uniform SPMD program processes key-tiles 0..4p+3 for pair p on every core;
which entries are causally masked is pure per-core DATA (the strip input).

Precision strategy (tolerance 2e-2; fp8 DoubleRow matmuls are 4x f32r rate,
bf16 is 2x, in the grading cost model):
  - All weights are pre-scaled by 32 on the host so that both fp8(32W) and
    the fp8 residual fp8(32W - fp8(32W)) sit well above e4m3's minimum
    subnormal (2^-9) — unscaled, |W|<=0.036 makes the residual term flush
    to zero. Projections run as 3-term fp8 DoubleRow hi/lo splits
    (x_hi@W_hi + x_hi@W_lo + x_lo@W_hi), giving ~bf16 accuracy at 75% of
    bf16 PE cost. The x32 scaling cancels: q,k stay scaled (32q, 32k; the
    1024x on scores folds into the exp scale constant), v is unscaled by
    1/32 during psum evacuation (a scaled copy, same cost).
  - QK^T scores: fp8 DoubleRow on fp8-cast 32q/32k (|32q| <= ~130 < 240).
    The only score noise is the fp8 cast; softmax normalization cancels
    common-mode and peaked rows are insensitive. Measured 1.35e-2.
  - attn@V context: bf16 (early causal rows copy v rows verbatim), with a
    ones-column appended to V so the softmax denominator falls out of the
    same matmul (exact normalization even after quantization).

Scores are computed TRANSPOSED (S^T = K Q^T with d on the contraction
partitions): the exp result in [key, query] layout feeds the context matmul
directly as the stationary operand — no PE transposes at all. Causal masking
is done ON THE PE: the last accumulation step of a diagonal tile's psum group
is matmul(diag(-1e30), strip01), adding -1e30 wherever strip==1. Two key
tiles share each [128,512] scores psum so one Activation exp serves both.

Schedule: phase order K -> V -> Q -> attention. K runs three passes over six
open psum groups so its first pass only waits on wk + x_hi chunk0; the
DMA-heavy/PE-light Q inputs stream in the shadow of K/V. DMA count is kept
low (descriptor prep is the bottleneck: one shared HWDGE device at ~625ns/DMA
for SP/Act/DVE queues, Pool software-DGE at ~1038ns/DMA) with a 2:1
sync:gpsimd split. The attention loop is software-pipelined (scores/exp run
4 key-tiles ahead of the context accumulation).
"""

import os
import sys
from contextlib import ExitStack

import numpy as np

for _p in ("/opt/trn_rl_repo", "/root/.axon_site/_ro/trn_rl_repo"):
    if os.path.isdir(_p) and _p not in sys.path:
        sys.path.append(_p)

import ml_dtypes  # noqa: E402

import concourse.mybir as mybir  # noqa: E402
import concourse.tile as tile  # noqa: E402
from concourse import bacc  # noqa: E402
from concourse.bass_utils import run_bass_kernel_spmd  # noqa: E402

F32 = mybir.dt.float32
BF16 = mybir.dt.bfloat16
FP8 = mybir.dt.float8e4
NP_FP8 = ml_dtypes.float8_e4m3
NP_BF16 = ml_dtypes.bfloat16
DR = mybir.MatmulPerfMode.DoubleRow
EXP = mybir.ActivationFunctionType.Exp

BATCH = 4
SEQ = 2048
D = 768
DK = D // 128  # 6 contraction k-tiles; 3 DoubleRow pairs
NQ = 1024  # query rows per core
WS = 32.0  # host-side weight pre-scale
ESCALE = float(1.0 / (np.sqrt(np.float32(D)) * WS * WS))
IVS = float(1.0 / WS)
NEG = -1e30

_CACHE = {}


def _build():
    nc = bacc.Bacc("TRN2", target_bir_lowering=False, debug=False, num_devices=8)
    xt_d = nc.declare_dram_parameter("xt", [D, SEQ], FP8, isOutput=False)
    xlo_d = nc.declare_dram_parameter("xlo", [D, SEQ], FP8, isOutput=False)
    xqt_d = nc.declare_dram_parameter("xqt", [D, NQ], FP8, isOutput=False)
    wq_d = nc.declare_dram_parameter("wq", [D, D], FP8, isOutput=False)
    wql_d = nc.declare_dram_parameter("wql", [D, D], FP8, isOutput=False)
    wk_d = nc.declare_dram_parameter("wk", [D, D], FP8, isOutput=False)
    wkl_d = nc.declare_dram_parameter("wkl", [D, D], FP8, isOutput=False)
    wvh_d = nc.declare_dram_parameter("wvh", [D, D], FP8, isOutput=False)
    wvl_d = nc.declare_dram_parameter("wvl", [D, D], FP8, isOutput=False)
    strip_d = nc.declare_dram_parameter("strip", [128, 512], BF16, isOutput=False)
    negd_d = nc.declare_dram_parameter("negd", [128, 128], BF16, isOutput=False)
    out_d = nc.declare_dram_parameter("out", [NQ, D], F32, isOutput=True)

    # 2:1 split between the SP HWDGE queue and the Pool SWDGE queue: one
    # shared HWDGE device serves SP/Act/DVE at ~625ns/DMA prep; Pool preps in
    # software (~1038ns) but on its own engine, in parallel. Emission order is
    # the prefetch schedule (DMA transfers serialize on the DMA engines).
    _dma_i = [0]

    def dma_in(dst, src):
        eng = (nc.sync, nc.gpsimd, nc.sync)[_dma_i[0] % 3]
        eng.dma_start(dst, src)
        _dma_i[0] += 1

    # Psum evacuations alternate DVE / Act; V variant fuses the 1/32 unscale.
    _evac_i = [0]

    def evac(dst, src, scale=None):
        if _evac_i[0] % 2 == 0:
            if scale is None:
                nc.vector.tensor_copy(dst, src)
            else:
                nc.vector.tensor_scalar_mul(dst, src, scale)
        else:
            if scale is None:
                nc.scalar.copy(dst, src)
            else:
                nc.scalar.mul(dst, src, scale)
        _evac_i[0] += 1

    def rearr(dram_slice):
        return dram_slice.rearrange("(ko p) s -> p ko s", p=128)

    def xsl(chunk, j, cs):
        """ko-pair j view of an x chunk: list of 3 [128,2,512] tiles, or one
        monolithic [128,6,512] tile."""
        if isinstance(chunk, list):
            return chunk[j][:, :, cs]
        return chunk[:, 2 * j : 2 * j + 2, cs]

    FULL = slice(0, 512)

    with tile.TileContext(nc) as tc, ExitStack() as ctx:
        persist = ctx.enter_context(tc.tile_pool(name="persist", bufs=1))

        qth = [persist.tile([128, DK, 512], FP8, name=f"qth{i}") for i in range(2)]
        kt = persist.tile([128, DK, SEQ], FP8)  # K^T resident (fp8 cast, 32-scaled)
        vtc = [persist.tile([128, 4, 769], BF16, name=f"vtc{i}") for i in range(4)]
        strip = persist.tile([128, 512], BF16)
        negd = persist.tile([128, 128], BF16)

        with ExitStack() as pw:
            xp = pw.enter_context(tc.tile_pool(name="xp", bufs=1))
            wkp = pw.enter_context(tc.tile_pool(name="wkp", bufs=1))
            wvp = pw.enter_context(tc.tile_pool(name="wvp", bufs=1))
            wqp = pw.enter_context(tc.tile_pool(name="wqp", bufs=1))
            xqp = pw.enter_context(tc.tile_pool(name="xqp", bufs=1))

            # ---------------- Phase K: K^T projection ----------------
            # Three passes over six open psum groups: pass 1 (wk * x_hi) only
            # needs wk + chunk0_hi, so the PE starts after ~1MB of DMA and the
            # pass-2/3 operands stream in behind it.
            with ExitStack() as pK:
                psk = pK.enter_context(tc.tile_pool(name="psk", bufs=6, space="PSUM"))
                wk = [wkp.tile([128, 2, D], FP8, name=f"wk{j}") for j in range(3)]
                wkl = [wkp.tile([128, 2, D], FP8, name=f"wkl{j}") for j in range(3)]
                xt8c = [
                    [xp.tile([128, 2, 512], FP8, name=f"xt8c0_{j}") for j in range(3)]
                ]
                xlo8c = [
                    [xp.tile([128, 2, 512], FP8, name=f"xlo8c0_{j}") for j in range(3)]
                ]
                for sc in range(1, 4):
                    xt8c.append(xp.tile([128, DK, 512], FP8, name=f"xt8c{sc}"))
                    xlo8c.append(xp.tile([128, DK, 512], FP8, name=f"xlo8c{sc}"))
                # pass-1 operands first, j-interleaved
                for j in range(3):
                    dma_in(wk[j][:], rearr(wk_d[j * 256 : (j + 1) * 256, :]))
                    dma_in(xt8c[0][j][:], rearr(xt_d[j * 256 : (j + 1) * 256, 0:512]))
                for j in range(3):
                    dma_in(wkl[j][:], rearr(wkl_d[j * 256 : (j + 1) * 256, :]))

                for sc in range(4):
                    if sc >= 1:
                        dma_in(xt8c[sc][:], rearr(xt_d[:, sc * 512 : sc * 512 + 512]))
                        dma_in(xlo8c[sc][:], rearr(xlo_d[:, sc * 512 : sc * 512 + 512]))
                    if sc == 2:
                        wvh = wvp.tile([128, DK, D], FP8, name="wvh")
                        wvl = wvp.tile([128, DK, D], FP8, name="wvl")
                        dma_in(wvh[:], rearr(wvh_d[:]))
                        dma_in(wvl[:], rearr(wvl_d[:]))
                    if sc == 2:
                        for j in range(3):
                            dma_in(xlo8c[0][j][:], rearr(xlo_d[j * 256 : (j + 1) * 256, 0:512]))
                    pks = [psk.tile([128, 512], F32, tag="psk", name=f"pks{_oo}") for _oo in range(DK)]
                    for pi, (wop, xop) in enumerate(((wk, xt8c[sc]), (wkl, xt8c[sc]))):
                        for oo in range(DK):
                            for j in range(3):
                                nc.tensor.matmul(
                                    pks[oo][:],
                                    wop[j][:, :, oo * 128 : oo * 128 + 128],
                                    xsl(xop, j, FULL),
                                    perf_mode=DR,
                                    start=(pi == 0 and j == 0),
                                    stop=(pi == 1 and j == 2),
                                )
                    for oo in range(DK):
                        evac(kt[:, oo, sc * 512 : sc * 512 + 512], pks[oo][:])

            # ---------------- Phase V: V projection ----------------
            with ExitStack() as pV:
                psv = pV.enter_context(tc.tile_pool(name="psv", bufs=3, space="PSUM"))
                xq8 = xqp.tile([128, DK, NQ], FP8, name="xq8")
                wq = wqp.tile([128, DK, D], FP8, name="wq")
                wql = wqp.tile([128, DK, D], FP8, name="wql")
                dma_in(wq[:], rearr(wq_d[:]))
                dma_in(xq8[:], rearr(xqt_d[:]))
                dma_in(wql[:], rearr(wql_d[:]))
                dma_in(strip[:], strip_d[:])
                dma_in(negd[:], negd_d[:])
                for sc in range(4):
                    nc.vector.memset(vtc[sc][:, :, 768:769], 1.0)
                    for st in range(4):
                        stc = slice(st * 128, st * 128 + 128)
                        for oc in range(2):
                            pv = psv.tile([128, 384], F32, tag="psv")
                            n = 0
                            for xop, wop in (
                                (xt8c[sc], wvh),
                                (xt8c[sc], wvl),
                                (xlo8c[sc], wvh),
                            ):
                                for j in range(3):
                                    nc.tensor.matmul(
                                        pv[:],
                                        xsl(xop, j, stc),
                                        wop[:, 2 * j : 2 * j + 2, oc * 384 : oc * 384 + 384],
                                        perf_mode=DR,
                                        start=(n == 0),
                                        stop=(n == 8),
                                    )
                                    n += 1
                            evac(vtc[sc][:, st, oc * 384 : oc * 384 + 384], pv[:], scale=IVS)

            # ---------------- Phase Q: Q^T projection ----------------
            with ExitStack() as pQ:
                psq = pQ.enter_context(tc.tile_pool(name="psq", bufs=3, space="PSUM"))
                for sc in range(2):
                    scc = slice(sc * 512, sc * 512 + 512)
                    for oo in range(DK):
                        pq = psq.tile([128, 512], F32, tag="psq")
                        n = 0
                        for wop in (wq, wql):
                            for j in range(3):
                                nc.tensor.matmul(
                                    pq[:],
                                    wop[:, 2 * j : 2 * j + 2, oo * 128 : oo * 128 + 128],
                                    xq8[:, 2 * j : 2 * j + 2, scc],
                                    perf_mode=DR,
                                    start=(n == 0),
                                    stop=(n == 5),
                                )
                                n += 1
                        evac(qth[sc][:, oo, :], pq[:])

        # ---------------- Phase A: attention (S^T scheme) ----------------
        with ExitStack() as pA:
            pss_p = pA.enter_context(tc.tile_pool(name="pss", bufs=2, space="PSUM"))
            pc1a_p = pA.enter_context(tc.tile_pool(name="pc1a", bufs=2, space="PSUM"))
            pc2a_p = pA.enter_context(tc.tile_pool(name="pc2a", bufs=1, space="PSUM"))
            pc1b_p = pA.enter_context(tc.tile_pool(name="pc1b", bufs=2, space="PSUM"))
            pc2b_p = pA.enter_context(tc.tile_pool(name="pc2b", bufs=1, space="PSUM"))
            attn_p = pA.enter_context(tc.tile_pool(name="attn", bufs=4))
            out_p = pA.enter_context(tc.tile_pool(name="outp", bufs=2))
            small_p = pA.enter_context(tc.tile_pool(name="small", bufs=2))

            LOOK = 2  # pipeline depth in steps (1 step = 2 key-tiles)

            def emit_scores(p, s):
                """Scores+exp for key-tiles (2s, 2s+1) of pair p; one shared
                [128,512] psum, one exp."""
                pss = pss_p.tile([128, 512], F32, tag="pss")
                for half in range(2):
                    kt_i = 2 * s + half
                    di = kt_i - 4 * p
                    hc = slice(half * 256, half * 256 + 256)
                    for j in range(3):
                        nc.tensor.matmul(
                            pss[:, hc],
                            kt[:, 2 * j : 2 * j + 2, kt_i * 128 : kt_i * 128 + 128],
                            qth[p // 2][
                                :, 2 * j : 2 * j + 2,
                                (p % 2) * 256 : (p % 2) * 256 + 256,
                            ],
                            perf_mode=DR,
                            start=(j == 0),
                            stop=(j == 2 and di < 0),
                        )
                    if di >= 0:
                        # causal mask on the PE: psum += -1e30 * strip01.
                        # Only one q-block can need masking at offset di
                        # (block0 for di<2, block1 for di>=2); which CORE
                        # masks is encoded in the strip data.
                        blkpos = 0 if di < 2 else 1
                        nc.tensor.matmul(
                            pss[:, half * 256 + blkpos * 128 : half * 256 + blkpos * 128 + 128],
                            negd[:],
                            strip[:, di * 128 : di * 128 + 128],
                            start=False,
                            stop=True,
                        )
                at = attn_p.tile([128, 512], BF16, tag="attn")
                nc.scalar.activation(at[:], pss[:], EXP, scale=ESCALE)
                return at

            def emit_ctx(p, s, at, pc1, pc2, nkt):
                for half in range(2):
                    kt_i = 2 * s + half
                    off = half * 256
                    for blk in range(2):
                        if kt_i >= nkt[blk]:
                            continue
                        last = kt_i == nkt[blk] - 1
                        lhsT = at[:, off + blk * 128 : off + blk * 128 + 128]
                        nc.tensor.matmul(
                            pc1[blk][:],
                            lhsT,
                            vtc[kt_i // 4][:, kt_i % 4, 0:512],
                            start=(kt_i == 0),
                            stop=last,
                        )
                        nc.tensor.matmul(
                            pc2[blk][:],
                            lhsT,
                            vtc[kt_i // 4][:, kt_i % 4, 512:769],
                            start=(kt_i == 0),
                            stop=last,
                        )
                        if last:
                            # normalize + store this q-block immediately; the
                            # two column halves use different engines (DVE /
                            # Act); out DMAs ride the HWDGE (sync) queue.
                            rinv = small_p.tile([128, 1], F32, tag="rinv")
                            nc.vector.reciprocal(rinv[:], pc2[blk][:, 256:257])
                            r = (2 * p + blk) * 128
                            osb1 = out_p.tile([128, 512], F32, tag="osb1")
                            nc.vector.tensor_mul(
                                osb1[:], pc1[blk][:], rinv[:].to_broadcast((128, 512))
                            )
                            nc.sync.dma_start(out_d[r : r + 128, 0:512], osb1[:])
                            osb2 = out_p.tile([128, 256], F32, tag="osb2")
                            nc.scalar.mul(osb2[:], pc2[blk][:, 0:256], rinv[:])
                            nc.gpsimd.dma_start(out_d[r : r + 128, 512:768], osb2[:])

            for p in range(4):
                pc1 = [
                    pc1a_p.tile([128, 512], F32, name="pc1a"),
                    pc1b_p.tile([128, 512], F32, name="pc1b"),
                ]
                pc2 = [
                    pc2a_p.tile([128, 257], F32, name="pc2a"),
                    pc2b_p.tile([128, 257], F32, name="pc2b"),
                ]
                nkt = [4 * p + 2, 4 * p + 4]  # ctx key-tile count per q-block
                S = 2 * p + 2  # steps (2 key-tiles each)
                ats = {}
                for it in range(S + LOOK):
                    if it < S:
                        ats[it] = emit_scores(p, it)
                    sc = it - LOOK
                    if sc >= 0:
                        emit_ctx(p, sc, ats.pop(sc), pc1, pc2, nkt)

    nc.compile()
    return nc


def _make_strip(h):
    """[128, 512] 0/1 mask; block i (128 wide) is added (via -1e30) to the
    masked q-block at diagonal offset i = kt - 4p. [key-row, query-col]."""
    tri = (np.arange(128)[:, None] > np.arange(128)[None, :]).astype(np.float32)
    ones = np.ones((128, 128), np.float32)
    zeros = np.zeros((128, 128), np.float32)
    blocks = [tri, ones, tri, ones] if h == 0 else [zeros, tri, zeros, tri]
    return np.concatenate(blocks, axis=1)


def _hi_lo(a):
    hi = a.astype(NP_FP8)
    lo = (a - hi.astype(np.float32)).astype(NP_FP8)
    return hi, lo


def kernel(x, Wq, Wk, Wv):
    if "nc" not in _CACHE:
        _CACHE["nc"] = _build()
    nc = _CACHE["nc"]

    x = np.ascontiguousarray(x, dtype=np.float32)
    wq8, wql8 = _hi_lo(WS * np.asarray(Wq, dtype=np.float32).T)
    wk8, wkl8 = _hi_lo(WS * np.asarray(Wk, dtype=np.float32).T)
    wvh8, wvl8 = _hi_lo(WS * np.asarray(Wv, dtype=np.float32).T)
    negd = (NEG * np.eye(128, dtype=np.float32)).astype(NP_BF16)

    in_maps = []
    for c in range(8):
        b, h = c // 2, c % 2
        xbt = np.ascontiguousarray(x[b].T)  # [768, 2048]
        xt8, xlo8 = _hi_lo(xbt)
        # own query columns: pairs p -> global tiles (4p+h, 4p+2+h)
        cols = []
        for p in range(4):
            for g in (4 * p + h, 4 * p + 2 + h):
                cols.append(xbt[:, g * 128 : (g + 1) * 128])
        xqt8 = np.ascontiguousarray(np.concatenate(cols, axis=1)).astype(NP_FP8)
        in_maps.append(
            {
                "xt": xt8,
                "xlo": xlo8,
                "xqt": xqt8,
                "wq": wq8,
                "wql": wql8,
                "wk": wk8,
                "wkl": wkl8,
                "wvh": wvh8,
                "wvl": wvl8,
                "strip": _make_strip(h).astype(NP_BF16),
                "negd": negd,
            }
        )

    res = run_bass_kernel_spmd(
        nc,
        in_maps,
        list(range(8)),
        trace=bool(int(os.environ.get("KERNEL_TRACE", "0"))),
    )
    _CACHE["last_results"] = res

    out = np.empty((BATCH, SEQ, D), np.float32)
    for c in range(8):
        b, h = c // 2, c % 2
        o = res.results[c]["out"]
        for p in range(4):
            for blk, g in enumerate((4 * p + h, 4 * p + 2 + h)):
                out[b, g * 128 : (g + 1) * 128] = o[
                    (2 * p + blk) * 128 : (2 * p + blk + 1) * 128
                ]
    return out


# revision 14
# speedup vs baseline: 1.8302x; 1.0110x over previous
"""Causal single-head attention on 8 TRN2 NeuronCores — fp8/bf16 edition.

Problem: x [4, 2048, 768] f32; Wq/Wk/Wv [768, 768] f32 (torch Linear layout).
  q/k/v = x @ W.T ; scores = q k^T causal-masked; attn = softmax(scores/sqrt(768));
  out = attn @ v.

Sharding: core c -> batch b = c//2, half h = c%2. Core h owns global q-tiles
{2lt+h}, grouped into 4 PAIRS: pair p = global tiles (4p+h, 4p+2+h). The
uniform SPMD program processes key-tiles 0..4p+3 for pair p on every core;
which entries are causally masked is pure per-core DATA (the strip input).

Precision strategy (tolerance 2e-2; fp8 DoubleRow matmuls are 4x f32r rate,
bf16 is 2x, in the grading cost model):
  - All weights are pre-scaled by 32 on the host so that both fp8(32W) and
    the fp8 residual fp8(32W - fp8(32W)) sit well above e4m3's minimum
    subnormal (2^-9) — unscaled, |W|<=0.036 makes the residual term flush
    to zero. Projections run as 3-term fp8 DoubleRow hi/lo splits
    (x_hi@W_hi + x_hi@W_lo + x_lo@W_hi), giving ~bf16 accuracy at 75% of
    bf16 PE cost. The x32 scaling cancels: q,k stay scaled (32q, 32k; the
    1024x on scores folds into the exp scale constant), v is unscaled by
    1/32 during psum evacuation (a scaled copy, same cost).
  - QK^T scores: fp8 DoubleRow on fp8-cast 32q/32k (|32q| <= ~130 < 240).
    The only score noise is the fp8 cast; softmax normalization cancels
    common-mode and peaked rows are insensitive. Measured 1.35e-2.
  - attn@V context: bf16 (early causal rows copy v rows verbatim), with a
    ones-column appended to V so the softmax denominator falls out of the
    same matmul (exact normalization even after quantization).

Scores are computed TRANSPOSED (S^T = K Q^T with d on the contraction
partitions): the exp result in [key, query] layout feeds the context matmul
directly as the stationary operand — no PE transposes at all. Causal masking
is done ON THE PE: the last accumulation step of a diagonal tile's psum group
is matmul(diag(-1e30), strip01), adding -1e30 wherever strip==1. Two key
tiles share each [128,512] scores psum so one Activation exp serves both.

Schedule: phase order K -> V -> Q -> attention. K runs three passes over six
open psum groups so its first pass only waits on wk + x_hi chunk0; the
DMA-heavy/PE-light Q inputs stream in the shadow of K/V. DMA count is kept
low (descriptor prep is the bottleneck: one shared HWDGE device at ~625ns/DMA
for SP/Act/DVE queues, Pool software-DGE at ~1038ns/DMA) with a 2:1
sync:gpsimd split. The attention loop is software-pipelined (scores/exp run
4 key-tiles ahead of the context accumulation).
"""

import os
import sys
from contextlib import ExitStack

import numpy as np

for _p in ("/opt/trn_rl_repo", "/root/.axon_site/_ro/trn_rl_repo"):
    if os.path.isdir(_p) and _p not in sys.path:
        sys.path.append(_p)

import ml_dtypes  # noqa: E402

import concourse.mybir as mybir  # noqa: E402
import concourse.tile as tile  # noqa: E402
from concourse import bacc  # noqa: E402
from concourse.bass_utils import run_bass_kernel_spmd  # noqa: E402

F32 = mybir.dt.float32
BF16 = mybir.dt.bfloat16
FP8 = mybir.dt.float8e4
NP_FP8 = ml_dtypes.float8_e4m3
NP_BF16 = ml_dtypes.bfloat16
DR = mybir.MatmulPerfMode.DoubleRow
EXP = mybir.ActivationFunctionType.Exp

BATCH = 4
SEQ = 2048
D = 768
DK = D // 128  # 6 contraction k-tiles; 3 DoubleRow pairs
NQ = 1024  # query rows per core
WS = 32.0  # host-side weight pre-scale
ESCALE = float(1.0 / (np.sqrt(np.float32(D)) * WS * WS))
IVS = float(1.0 / WS)
NEG = -1e30

_CACHE = {}


def _build():
    nc = bacc.Bacc("TRN2", target_bir_lowering=False, debug=False, num_devices=8)
    xt_d = nc.declare_dram_parameter("xt", [D, SEQ], FP8, isOutput=False)
    xlo_d = nc.declare_dram_parameter("xlo", [D, SEQ], FP8, isOutput=False)
    xqt_d = nc.declare_dram_parameter("xqt", [D, NQ], FP8, isOutput=False)
    wq_d = nc.declare_dram_parameter("wq", [D, D], FP8, isOutput=False)
    wql_d = nc.declare_dram_parameter("wql", [D, D], FP8, isOutput=False)
    wk_d = nc.declare_dram_parameter("wk", [D, D], FP8, isOutput=False)
    wkl_d = nc.declare_dram_parameter("wkl", [D, D], FP8, isOutput=False)
    wvh_d = nc.declare_dram_parameter("wvh", [D, D], FP8, isOutput=False)
    wvl_d = nc.declare_dram_parameter("wvl", [D, D], FP8, isOutput=False)
    strip_d = nc.declare_dram_parameter("strip", [128, 512], BF16, isOutput=False)
    negd_d = nc.declare_dram_parameter("negd", [128, 128], BF16, isOutput=False)
    out_d = nc.declare_dram_parameter("out", [NQ, D], F32, isOutput=True)

    # 2:1 split between the SP HWDGE queue and the Pool SWDGE queue: one
    # shared HWDGE device serves SP/Act/DVE at ~625ns/DMA prep; Pool preps in
    # software (~1038ns) but on its own engine, in parallel. Emission order is
    # the prefetch schedule (DMA transfers serialize on the DMA engines).
    _dma_i = [0]

    def dma_in(dst, src):
        eng = (nc.sync, nc.gpsimd, nc.sync)[_dma_i[0] % 3]
        eng.dma_start(dst, src)
        _dma_i[0] += 1

    # Psum evacuations alternate DVE / Act; V variant fuses the 1/32 unscale.
    _evac_i = [0]

    def evac(dst, src, scale=None):
        if _evac_i[0] % 2 == 0:
            if scale is None:
                nc.vector.tensor_copy(dst, src)
            else:
                nc.vector.tensor_scalar_mul(dst, src, scale)
        else:
            if scale is None:
                nc.scalar.copy(dst, src)
            else:
                nc.scalar.mul(dst, src, scale)
        _evac_i[0] += 1

    def rearr(dram_slice):
        return dram_slice.rearrange("(ko p) s -> p ko s", p=128)

    def xsl(chunk, j, cs):
        """ko-pair j view of an x chunk: list of 3 [128,2,512] tiles, or one
        monolithic [128,6,512] tile."""
        if isinstance(chunk, list):
            return chunk[j][:, :, cs]
        return chunk[:, 2 * j : 2 * j + 2, cs]

    FULL = slice(0, 512)

    with tile.TileContext(nc) as tc, ExitStack() as ctx:
        persist = ctx.enter_context(tc.tile_pool(name="persist", bufs=1))

        qth = [persist.tile([128, DK, 512], FP8, name=f"qth{i}") for i in range(2)]
        kt = persist.tile([128, DK, SEQ], FP8)  # K^T resident (fp8 cast, 32-scaled)
        vtc = [persist.tile([128, 4, 769], BF16, name=f"vtc{i}") for i in range(4)]
        strip = persist.tile([128, 512], BF16)
        negd = persist.tile([128, 128], BF16)

        with ExitStack() as pw:
            xp = pw.enter_context(tc.tile_pool(name="xp", bufs=1))
            wkp = pw.enter_context(tc.tile_pool(name="wkp", bufs=1))
            wvp = pw.enter_context(tc.tile_pool(name="wvp", bufs=1))
            wqp = pw.enter_context(tc.tile_pool(name="wqp", bufs=1))
            xqp = pw.enter_context(tc.tile_pool(name="xqp", bufs=1))

            # ---------------- Phase K: K^T projection ----------------
            # Three passes over six open psum groups: pass 1 (wk * x_hi) only
            # needs wk + chunk0_hi, so the PE starts after ~1MB of DMA and the
            # pass-2/3 operands stream in behind it.
            with ExitStack() as pK:
                psk = pK.enter_context(tc.tile_pool(name="psk", bufs=6, space="PSUM"))
                wk = [wkp.tile([128, 2, D], FP8, name=f"wk{j}") for j in range(3)]
                wkl = [wkp.tile([128, 2, D], FP8, name=f"wkl{j}") for j in range(3)]
                xt8c = [
                    [xp.tile([128, 2, 512], FP8, name=f"xt8c0_{j}") for j in range(3)]
                ]
                xlo8c = [
                    [xp.tile([128, 2, 512], FP8, name=f"xlo8c0_{j}") for j in range(3)]
                ]
                for sc in range(1, 4):
                    xt8c.append(xp.tile([128, DK, 512], FP8, name=f"xt8c{sc}"))
                    xlo8c.append(xp.tile([128, DK, 512], FP8, name=f"xlo8c{sc}"))
                # pass-1 operands first, j-interleaved
                for j in range(3):
                    dma_in(wk[j][:], rearr(wk_d[j * 256 : (j + 1) * 256, :]))
                    dma_in(xt8c[0][j][:], rearr(xt_d[j * 256 : (j + 1) * 256, 0:512]))
                for j in range(3):
                    dma_in(wkl[j][:], rearr(wkl_d[j * 256 : (j + 1) * 256, :]))

                for sc in range(4):
                    if sc >= 1:
                        dma_in(xt8c[sc][:], rearr(xt_d[:, sc * 512 : sc * 512 + 512]))
                        dma_in(xlo8c[sc][:], rearr(xlo_d[:, sc * 512 : sc * 512 + 512]))
                    if sc == 3:
                        for j in range(3):
                            dma_in(xlo8c[0][j][:], rearr(xlo_d[j * 256 : (j + 1) * 256, 0:512]))
                    if sc == 2:
                        wvh = wvp.tile([128, DK, D], FP8, name="wvh")
                        wvl = wvp.tile([128, DK, D], FP8, name="wvl")
                        dma_in(wvh[:], rearr(wvh_d[:]))
                        dma_in(wvl[:], rearr(wvl_d[:]))
                    pks = [psk.tile([128, 512], F32, tag="psk", name=f"pks{_oo}") for _oo in range(DK)]
                    for pi, (wop, xop) in enumerate(((wk, xt8c[sc]), (wkl, xt8c[sc]))):
                        for oo in range(DK):
                            for j in range(3):
                                nc.tensor.matmul(
                                    pks[oo][:],
                                    wop[j][:, :, oo * 128 : oo * 128 + 128],
                                    xsl(xop, j, FULL),
                                    perf_mode=DR,
                                    start=(pi == 0 and j == 0),
                                    stop=(pi == 1 and j == 2),
                                )
                    for oo in range(DK):
                        evac(kt[:, oo, sc * 512 : sc * 512 + 512], pks[oo][:])

            # ---------------- Phase V: V projection ----------------
            with ExitStack() as pV:
                psv = pV.enter_context(tc.tile_pool(name="psv", bufs=3, space="PSUM"))
                xq8 = xqp.tile([128, DK, NQ], FP8, name="xq8")
                wq = wqp.tile([128, DK, D], FP8, name="wq")
                wql = wqp.tile([128, DK, D], FP8, name="wql")
                dma_in(wq[:], rearr(wq_d[:]))
                dma_in(xq8[:], rearr(xqt_d[:]))
                dma_in(wql[:], rearr(wql_d[:]))
                dma_in(strip[:], strip_d[:])
                dma_in(negd[:], negd_d[:])
                for sc in range(4):
                    nc.vector.memset(vtc[sc][:, :, 768:769], 1.0)
                    for st in range(4):
                        stc = slice(st * 128, st * 128 + 128)
                        for oc in range(2):
                            pv = psv.tile([128, 384], F32, tag="psv")
                            n = 0
                            for xop, wop in (
                                (xt8c[sc], wvh),
                                (xt8c[sc], wvl),
                                (xlo8c[sc], wvh),
                            ):
                                for j in range(3):
                                    nc.tensor.matmul(
                                        pv[:],
                                        xsl(xop, j, stc),
                                        wop[:, 2 * j : 2 * j + 2, oc * 384 : oc * 384 + 384],
                                        perf_mode=DR,
                                        start=(n == 0),
                                        stop=(n == 8),
                                    )
                                    n += 1
                            evac(vtc[sc][:, st, oc * 384 : oc * 384 + 384], pv[:], scale=IVS)

            # ---------------- Phase Q: Q^T projection ----------------
            with ExitStack() as pQ:
                psq = pQ.enter_context(tc.tile_pool(name="psq", bufs=3, space="PSUM"))
                for sc in range(2):
                    scc = slice(sc * 512, sc * 512 + 512)
                    for oo in range(DK):
                        pq = psq.tile([128, 512], F32, tag="psq")
                        n = 0
                        for wop in (wq, wql):
                            for j in range(3):
                                nc.tensor.matmul(
                                    pq[:],
                                    wop[:, 2 * j : 2 * j + 2, oo * 128 : oo * 128 + 128],
                                    xq8[:, 2 * j : 2 * j + 2, scc],
                                    perf_mode=DR,
                                    start=(n == 0),
                                    stop=(n == 5),
                                )
                                n += 1
                        evac(qth[sc][:, oo, :], pq[:])

        # ---------------- Phase A: attention (S^T scheme) ----------------
        with ExitStack() as pA:
            pss_p = pA.enter_context(tc.tile_pool(name="pss", bufs=2, space="PSUM"))
            pc1a_p = pA.enter_context(tc.tile_pool(name="pc1a", bufs=1, space="PSUM"))
            pc2a_p = pA.enter_context(tc.tile_pool(name="pc2a", bufs=2, space="PSUM"))
            pc1b_p = pA.enter_context(tc.tile_pool(name="pc1b", bufs=1, space="PSUM"))
            pc2b_p = pA.enter_context(tc.tile_pool(name="pc2b", bufs=2, space="PSUM"))
            attn_p = pA.enter_context(tc.tile_pool(name="attn", bufs=4))
            out_p = pA.enter_context(tc.tile_pool(name="outp", bufs=2))
            small_p = pA.enter_context(tc.tile_pool(name="small", bufs=2))

            LOOK = 2  # pipeline depth in steps (1 step = 2 key-tiles)

            def emit_scores(p, s):
                """Scores+exp for key-tiles (2s, 2s+1) of pair p; one shared
                [128,512] psum, one exp."""
                pss = pss_p.tile([128, 512], F32, tag="pss")
                for half in range(2):
                    kt_i = 2 * s + half
                    di = kt_i - 4 * p
                    hc = slice(half * 256, half * 256 + 256)
                    for j in range(3):
                        nc.tensor.matmul(
                            pss[:, hc],
                            kt[:, 2 * j : 2 * j + 2, kt_i * 128 : kt_i * 128 + 128],
                            qth[p // 2][
                                :, 2 * j : 2 * j + 2,
                                (p % 2) * 256 : (p % 2) * 256 + 256,
                            ],
                            perf_mode=DR,
                            start=(j == 0),
                            stop=(j == 2 and di < 0),
                        )
                    if di >= 0:
                        # causal mask on the PE: psum += -1e30 * strip01.
                        # Only one q-block can need masking at offset di
                        # (block0 for di<2, block1 for di>=2); which CORE
                        # masks is encoded in the strip data.
                        blkpos = 0 if di < 2 else 1
                        nc.tensor.matmul(
                            pss[:, half * 256 + blkpos * 128 : half * 256 + blkpos * 128 + 128],
                            negd[:],
                            strip[:, di * 128 : di * 128 + 128],
                            start=False,
                            stop=True,
                        )
                at = attn_p.tile([128, 512], BF16, tag="attn")
                nc.scalar.activation(at[:], pss[:], EXP, scale=ESCALE)
                return at

            def emit_ctx(p, s, at, pc1, pc2, nkt):
                for half in range(2):
                    kt_i = 2 * s + half
                    off = half * 256
                    for blk in range(2):
                        if kt_i >= nkt[blk]:
                            continue
                        last = kt_i == nkt[blk] - 1
                        lhsT = at[:, off + blk * 128 : off + blk * 128 + 128]
                        nc.tensor.matmul(
                            pc1[blk][:],
                            lhsT,
                            vtc[kt_i // 4][:, kt_i % 4, 0:512],
                            start=(kt_i == 0),
                            stop=last,
                        )
                        nc.tensor.matmul(
                            pc2[blk][:],
                            lhsT,
                            vtc[kt_i // 4][:, kt_i % 4, 512:769],
                            start=(kt_i == 0),
                            stop=last,
                        )
                        if last:
                            # normalize + store this q-block immediately; the
                            # two column halves use different engines (DVE /
                            # Act); out DMAs ride the HWDGE (sync) queue.
                            rinv = small_p.tile([128, 1], F32, tag="rinv")
                            nc.vector.reciprocal(rinv[:], pc2[blk][:, 256:257])
                            r = (2 * p + blk) * 128
                            osb1 = out_p.tile([128, 512], F32, tag="osb1")
                            nc.vector.tensor_mul(
                                osb1[:], pc1[blk][:], rinv[:].to_broadcast((128, 512))
                            )
                            nc.sync.dma_start(out_d[r : r + 128, 0:512], osb1[:])
                            osb2 = out_p.tile([128, 256], F32, tag="osb2")
                            nc.scalar.mul(osb2[:], pc2[blk][:, 0:256], rinv[:])
                            nc.gpsimd.dma_start(out_d[r : r + 128, 512:768], osb2[:])

            for p in range(4):
                pc1 = [
                    pc1a_p.tile([128, 512], F32, name="pc1a"),
                    pc1b_p.tile([128, 512], F32, name="pc1b"),
                ]
                pc2 = [
                    pc2a_p.tile([128, 257], F32, name="pc2a"),
                    pc2b_p.tile([128, 257], F32, name="pc2b"),
                ]
                nkt = [4 * p + 2, 4 * p + 4]  # ctx key-tile count per q-block
                S = 2 * p + 2  # steps (2 key-tiles each)
                ats = {}
                for it in range(S + LOOK):
                    if it < S:
                        ats[it] = emit_scores(p, it)
                    sc = it - LOOK
                    if sc >= 0:
                        emit_ctx(p, sc, ats.pop(sc), pc1, pc2, nkt)

    nc.compile()
    return nc


def _make_strip(h):
    """[128, 512] 0/1 mask; block i (128 wide) is added (via -1e30) to the
    masked q-block at diagonal offset i = kt - 4p. [key-row, query-col]."""
    tri = (np.arange(128)[:, None] > np.arange(128)[None, :]).astype(np.float32)
    ones = np.ones((128, 128), np.float32)
    zeros = np.zeros((128, 128), np.float32)
    blocks = [tri, ones, tri, ones] if h == 0 else [zeros, tri, zeros, tri]
    return np.concatenate(blocks, axis=1)


def _hi_lo(a):
    hi = a.astype(NP_FP8)
    lo = (a - hi.astype(np.float32)).astype(NP_FP8)
    return hi, lo


def kernel(x, Wq, Wk, Wv):
    if "nc" not in _CACHE:
        _CACHE["nc"] = _build()
    nc = _CACHE["nc"]

    x = np.ascontiguousarray(x, dtype=np.float32)
    wq8, wql8 = _hi_lo(WS * np.asarray(Wq, dtype=np.float32).T)
    wk8, wkl8 = _hi_lo(WS * np.asarray(Wk, dtype=np.float32).T)
    wvh8, wvl8 = _hi_lo(WS * np.asarray(Wv, dtype=np.float32).T)
    negd = (NEG * np.eye(128, dtype=np.float32)).astype(NP_BF16)

    in_maps = []
    for c in range(8):
        b, h = c // 2, c % 2
        xbt = np.ascontiguousarray(x[b].T)  # [768, 2048]
        xt8, xlo8 = _hi_lo(xbt)
        # own query columns: pairs p -> global tiles (4p+h, 4p+2+h)
        cols = []
        for p in range(4):
            for g in (4 * p + h, 4 * p + 2 + h):
                cols.append(xbt[:, g * 128 : (g + 1) * 128])
        xqt8 = np.ascontiguousarray(np.concatenate(cols, axis=1)).astype(NP_FP8)
        in_maps.append(
            {
                "xt": xt8,
                "xlo": xlo8,
                "xqt": xqt8,
                "wq": wq8,
                "wql": wql8,
                "wk": wk8,
                "wkl": wkl8,
                "wvh": wvh8,
                "wvl": wvl8,
                "strip": _make_strip(h).astype(NP_BF16),
                "negd": negd,
            }
        )

    res = run_bass_kernel_spmd(
        nc,
        in_maps,
        list(range(8)),
        trace=bool(int(os.environ.get("KERNEL_TRACE", "0"))),
    )
    _CACHE["last_results"] = res

    out = np.empty((BATCH, SEQ, D), np.float32)
    for c in range(8):
        b, h = c // 2, c % 2
        o = res.results[c]["out"]
        for p in range(4):
            for blk, g in enumerate((4 * p + h, 4 * p + 2 + h)):
                out[b, g * 128 : (g + 1) * 128] = o[
                    (2 * p + blk) * 128 : (2 * p + blk + 1) * 128
                ]
    return out


# revision 15
# speedup vs baseline: 1.8730x; 1.0234x over previous
"""Causal single-head attention on 8 TRN2 NeuronCores — fp8/bf16 edition.

Problem: x [4, 2048, 768] f32; Wq/Wk/Wv [768, 768] f32 (torch Linear layout).
  q/k/v = x @ W.T ; scores = q k^T causal-masked; attn = softmax(scores/sqrt(768));
  out = attn @ v.

Sharding: core c -> batch b = c//2, half h = c%2. Core h owns global q-tiles
{2lt+h}, grouped into 4 PAIRS: pair p = global tiles (4p+h, 4p+2+h). The
uniform SPMD program processes key-tiles 0..4p+3 for pair p on every core;
which entries are causally masked is pure per-core DATA (the strip input).

Precision strategy (tolerance 2e-2; fp8 DoubleRow matmuls are 4x f32r rate,
bf16 is 2x, in the grading cost model):
  - All weights are pre-scaled by 32 on the host so that both fp8(32W) and
    the fp8 residual fp8(32W - fp8(32W)) sit well above e4m3's minimum
    subnormal (2^-9) — unscaled, |W|<=0.036 makes the residual term flush
    to zero. Projections run as 3-term fp8 DoubleRow hi/lo splits
    (x_hi@W_hi + x_hi@W_lo + x_lo@W_hi), giving ~bf16 accuracy at 75% of
    bf16 PE cost. The x32 scaling cancels: q,k stay scaled (32q, 32k; the
    1024x on scores folds into the exp scale constant), v is unscaled by
    1/32 during psum evacuation (a scaled copy, same cost).
  - QK^T scores: fp8 DoubleRow on fp8-cast 32q/32k (|32q| <= ~130 < 240).
    The only score noise is the fp8 cast; softmax normalization cancels
    common-mode and peaked rows are insensitive. Measured 1.35e-2.
  - attn@V context: bf16 (early causal rows copy v rows verbatim), with a
    ones-column appended to V so the softmax denominator falls out of the
    same matmul (exact normalization even after quantization).

Scores are computed TRANSPOSED (S^T = K Q^T with d on the contraction
partitions): the exp result in [key, query] layout feeds the context matmul
directly as the stationary operand — no PE transposes at all. Causal masking
is done ON THE PE: the last accumulation step of a diagonal tile's psum group
is matmul(diag(-1e30), strip01), adding -1e30 wherever strip==1. Two key
tiles share each [128,512] scores psum so one Activation exp serves both.

Schedule: phase order K -> V -> Q -> attention. K runs three passes over six
open psum groups so its first pass only waits on wk + x_hi chunk0; the
DMA-heavy/PE-light Q inputs stream in the shadow of K/V. DMA count is kept
low (descriptor prep is the bottleneck: one shared HWDGE device at ~625ns/DMA
for SP/Act/DVE queues, Pool software-DGE at ~1038ns/DMA) with a 2:1
sync:gpsimd split. The attention loop is software-pipelined (scores/exp run
4 key-tiles ahead of the context accumulation).
"""

import os
import sys
from contextlib import ExitStack

import numpy as np

for _p in ("/opt/trn_rl_repo", "/root/.axon_site/_ro/trn_rl_repo"):
    if os.path.isdir(_p) and _p not in sys.path:
        sys.path.append(_p)

import ml_dtypes  # noqa: E402

import concourse.mybir as mybir  # noqa: E402
import concourse.tile as tile  # noqa: E402
from concourse import bacc  # noqa: E402
from concourse.bass_utils import run_bass_kernel_spmd  # noqa: E402

F32 = mybir.dt.float32
BF16 = mybir.dt.bfloat16
FP8 = mybir.dt.float8e4
NP_FP8 = ml_dtypes.float8_e4m3
NP_BF16 = ml_dtypes.bfloat16
DR = mybir.MatmulPerfMode.DoubleRow
EXP = mybir.ActivationFunctionType.Exp

BATCH = 4
SEQ = 2048
D = 768
DK = D // 128  # 6 contraction k-tiles; 3 DoubleRow pairs
NQ = 1024  # query rows per core
WS = 32.0  # host-side weight pre-scale
ESCALE = float(1.0 / (np.sqrt(np.float32(D)) * WS * WS))
IVS = float(1.0 / WS)
NEG = -1e30

_CACHE = {}


def _build():
    nc = bacc.Bacc("TRN2", target_bir_lowering=False, debug=False, num_devices=8)
    xt_d = nc.declare_dram_parameter("xt", [D, SEQ], FP8, isOutput=False)
    xlo_d = nc.declare_dram_parameter("xlo", [D, SEQ], FP8, isOutput=False)
    xqt_d = nc.declare_dram_parameter("xqt", [D, NQ], FP8, isOutput=False)
    wq_d = nc.declare_dram_parameter("wq", [D, D], FP8, isOutput=False)
    wql_d = nc.declare_dram_parameter("wql", [D, D], FP8, isOutput=False)
    wk_d = nc.declare_dram_parameter("wk", [D, D], FP8, isOutput=False)
    wkl_d = nc.declare_dram_parameter("wkl", [D, D], FP8, isOutput=False)
    wvh_d = nc.declare_dram_parameter("wvh", [D, D], FP8, isOutput=False)
    wvl_d = nc.declare_dram_parameter("wvl", [D, D], FP8, isOutput=False)
    strip_d = nc.declare_dram_parameter("strip", [128, 512], BF16, isOutput=False)
    negd_d = nc.declare_dram_parameter("negd", [128, 128], BF16, isOutput=False)
    out_d = nc.declare_dram_parameter("out", [NQ, D], F32, isOutput=True)

    # 2:1 split between the SP HWDGE queue and the Pool SWDGE queue: one
    # shared HWDGE device serves SP/Act/DVE at ~625ns/DMA prep; Pool preps in
    # software (~1038ns) but on its own engine, in parallel. Emission order is
    # the prefetch schedule (DMA transfers serialize on the DMA engines).
    _dma_i = [0]

    def dma_in(dst, src):
        eng = (nc.sync, nc.gpsimd, nc.sync)[_dma_i[0] % 3]
        eng.dma_start(dst, src)
        _dma_i[0] += 1

    # Psum evacuations alternate DVE / Act; V variant fuses the 1/32 unscale.
    _evac_i = [0]

    def evac(dst, src, scale=None):
        if _evac_i[0] % 2 == 0:
            if scale is None:
                nc.vector.tensor_copy(dst, src)
            else:
                nc.vector.tensor_scalar_mul(dst, src, scale)
        else:
            if scale is None:
                nc.scalar.copy(dst, src)
            else:
                nc.scalar.mul(dst, src, scale)
        _evac_i[0] += 1

    def rearr(dram_slice):
        return dram_slice.rearrange("(ko p) s -> p ko s", p=128)

    def xsl(chunk, j, cs):
        """ko-pair j view of an x chunk: list of 3 [128,2,512] tiles, or one
        monolithic [128,6,512] tile."""
        if isinstance(chunk, list):
            return chunk[j][:, :, cs]
        return chunk[:, 2 * j : 2 * j + 2, cs]

    FULL = slice(0, 512)

    with tile.TileContext(nc) as tc, ExitStack() as ctx:
        persist = ctx.enter_context(tc.tile_pool(name="persist", bufs=1))

        qth = [persist.tile([128, DK, 512], FP8, name=f"qth{i}") for i in range(2)]
        kt = persist.tile([128, DK, SEQ], FP8)  # K^T resident (fp8 cast, 32-scaled)
        vtc = [persist.tile([128, 4, 769], BF16, name=f"vtc{i}") for i in range(4)]
        strip = persist.tile([128, 512], BF16)
        negd = persist.tile([128, 128], BF16)

        with ExitStack() as pw:
            xp = pw.enter_context(tc.tile_pool(name="xp", bufs=1))
            wkp = pw.enter_context(tc.tile_pool(name="wkp", bufs=1))
            wvp = pw.enter_context(tc.tile_pool(name="wvp", bufs=1))
            wqp = pw.enter_context(tc.tile_pool(name="wqp", bufs=1))
            xqp = pw.enter_context(tc.tile_pool(name="xqp", bufs=1))

            # ---------------- Phase K: K^T projection ----------------
            # Three passes over six open psum groups: pass 1 (wk * x_hi) only
            # needs wk + chunk0_hi, so the PE starts after ~1MB of DMA and the
            # pass-2/3 operands stream in behind it.
            with ExitStack() as pK:
                psk = pK.enter_context(tc.tile_pool(name="psk", bufs=6, space="PSUM"))
                wk = [wkp.tile([128, 2, D], FP8, name=f"wk{j}") for j in range(3)]
                wkl = [wkp.tile([128, 2, D], FP8, name=f"wkl{j}") for j in range(3)]
                xt8c = [
                    [xp.tile([128, 2, 512], FP8, name=f"xt8c0_{j}") for j in range(3)]
                ]
                xlo8c = [
                    [xp.tile([128, 2, 512], FP8, name=f"xlo8c0_{j}") for j in range(3)]
                ]
                for sc in range(1, 4):
                    xt8c.append(xp.tile([128, DK, 512], FP8, name=f"xt8c{sc}"))
                    xlo8c.append(xp.tile([128, DK, 512], FP8, name=f"xlo8c{sc}"))
                # pass-1 operands first, j-interleaved
                for j in range(3):
                    dma_in(wk[j][:], rearr(wk_d[j * 256 : (j + 1) * 256, :]))
                    dma_in(xt8c[0][j][:], rearr(xt_d[j * 256 : (j + 1) * 256, 0:512]))
                for j in range(3):
                    dma_in(wkl[j][:], rearr(wkl_d[j * 256 : (j + 1) * 256, :]))

                for sc in range(4):
                    if sc >= 1:
                        dma_in(xt8c[sc][:], rearr(xt_d[:, sc * 512 : sc * 512 + 512]))
                        dma_in(xlo8c[sc][:], rearr(xlo_d[:, sc * 512 : sc * 512 + 512]))
                    if sc == 3:
                        for j in range(3):
                            dma_in(xlo8c[0][j][:], rearr(xlo_d[j * 256 : (j + 1) * 256, 0:512]))
                    if sc == 2:
                        wvh = wvp.tile([128, DK, D], FP8, name="wvh")
                        wvl = wvp.tile([128, DK, D], FP8, name="wvl")
                        dma_in(wvh[:], rearr(wvh_d[:]))
                        dma_in(wvl[:], rearr(wvl_d[:]))
                    pks = [psk.tile([128, 512], F32, tag="psk", name=f"pks{_oo}") for _oo in range(DK)]
                    for pi, (wop, xop) in enumerate(((wk, xt8c[sc]), (wkl, xt8c[sc]))):
                        for oo in range(DK):
                            for j in range(3):
                                nc.tensor.matmul(
                                    pks[oo][:],
                                    wop[j][:, :, oo * 128 : oo * 128 + 128],
                                    xsl(xop, j, FULL),
                                    perf_mode=DR,
                                    start=(pi == 0 and j == 0),
                                    stop=(pi == 1 and j == 2),
                                )
                    for oo in range(DK):
                        evac(kt[:, oo, sc * 512 : sc * 512 + 512], pks[oo][:])

            # ---------------- Phase V: V projection ----------------
            with ExitStack() as pV:
                psv = pV.enter_context(tc.tile_pool(name="psv", bufs=3, space="PSUM"))
                xq8 = xqp.tile([128, DK, NQ], FP8, name="xq8")
                wq = wqp.tile([128, DK, D], FP8, name="wq")
                wql = wqp.tile([128, DK, D], FP8, name="wql")
                dma_in(wq[:], rearr(wq_d[:]))
                dma_in(xq8[:], rearr(xqt_d[:]))
                dma_in(wql[:], rearr(wql_d[:]))
                dma_in(strip[:], strip_d[:])
                dma_in(negd[:], negd_d[:])
                for sc in range(4):
                    nc.vector.memset(vtc[sc][:, :, 768:769], 1.0)
                    for st in range(4):
                        stc = slice(st * 128, st * 128 + 128)
                        for oc in range(2):
                            pv = psv.tile([128, 384], F32, tag="psv")
                            n = 0
                            for xop, wop in (
                                (xt8c[sc], wvh),
                                (xt8c[sc], wvl),
                                (xlo8c[sc], wvh),
                            ):
                                for j in range(3):
                                    nc.tensor.matmul(
                                        pv[:],
                                        xsl(xop, j, stc),
                                        wop[:, 2 * j : 2 * j + 2, oc * 384 : oc * 384 + 384],
                                        perf_mode=DR,
                                        start=(n == 0),
                                        stop=(n == 8),
                                    )
                                    n += 1
                            evac(vtc[sc][:, st, oc * 384 : oc * 384 + 384], pv[:], scale=IVS)

            # ---------------- Phase Q: Q^T projection ----------------
            with ExitStack() as pQ:
                psq = pQ.enter_context(tc.tile_pool(name="psq", bufs=3, space="PSUM"))
                for sc in range(2):
                    scc = slice(sc * 512, sc * 512 + 512)
                    for oo in range(DK):
                        pq = psq.tile([128, 512], F32, tag="psq")
                        n = 0
                        for wop in (wq, wql):
                            for j in range(3):
                                nc.tensor.matmul(
                                    pq[:],
                                    wop[:, 2 * j : 2 * j + 2, oo * 128 : oo * 128 + 128],
                                    xq8[:, 2 * j : 2 * j + 2, scc],
                                    perf_mode=DR,
                                    start=(n == 0),
                                    stop=(n == 5),
                                )
                                n += 1
                        evac(qth[sc][:, oo, :], pq[:])

        # ---------------- Phase A: attention (S^T scheme) ----------------
        with ExitStack() as pA:
            pss_p = pA.enter_context(tc.tile_pool(name="pss", bufs=3, space="PSUM"))
            pc1a_p = pA.enter_context(tc.tile_pool(name="pc1a", bufs=1, space="PSUM"))
            pc2a_p = pA.enter_context(tc.tile_pool(name="pc2a", bufs=1, space="PSUM"))
            pc1b_p = pA.enter_context(tc.tile_pool(name="pc1b", bufs=1, space="PSUM"))
            pc2b_p = pA.enter_context(tc.tile_pool(name="pc2b", bufs=1, space="PSUM"))
            attn_p = pA.enter_context(tc.tile_pool(name="attn", bufs=5))
            out_p = pA.enter_context(tc.tile_pool(name="outp", bufs=2))
            small_p = pA.enter_context(tc.tile_pool(name="small", bufs=2))

            LOOK = 3  # pipeline depth in steps (1 step = 2 key-tiles)

            def emit_scores(p, s):
                """Scores+exp for key-tiles (2s, 2s+1) of pair p; one shared
                [128,512] psum, one exp."""
                pss = pss_p.tile([128, 512], F32, tag="pss")
                for half in range(2):
                    kt_i = 2 * s + half
                    di = kt_i - 4 * p
                    hc = slice(half * 256, half * 256 + 256)
                    for j in range(3):
                        nc.tensor.matmul(
                            pss[:, hc],
                            kt[:, 2 * j : 2 * j + 2, kt_i * 128 : kt_i * 128 + 128],
                            qth[p // 2][
                                :, 2 * j : 2 * j + 2,
                                (p % 2) * 256 : (p % 2) * 256 + 256,
                            ],
                            perf_mode=DR,
                            start=(j == 0),
                            stop=(j == 2 and di < 0),
                        )
                    if di >= 0:
                        # causal mask on the PE: psum += -1e30 * strip01.
                        # Only one q-block can need masking at offset di
                        # (block0 for di<2, block1 for di>=2); which CORE
                        # masks is encoded in the strip data.
                        blkpos = 0 if di < 2 else 1
                        nc.tensor.matmul(
                            pss[:, half * 256 + blkpos * 128 : half * 256 + blkpos * 128 + 128],
                            negd[:],
                            strip[:, di * 128 : di * 128 + 128],
                            start=False,
                            stop=True,
                        )
                at = attn_p.tile([128, 512], BF16, tag="attn")
                nc.scalar.activation(at[:], pss[:], EXP, scale=ESCALE)
                return at

            def emit_ctx(p, s, at, pc1, pc2, nkt):
                for half in range(2):
                    kt_i = 2 * s + half
                    off = half * 256
                    for blk in range(2):
                        if kt_i >= nkt[blk]:
                            continue
                        last = kt_i == nkt[blk] - 1
                        lhsT = at[:, off + blk * 128 : off + blk * 128 + 128]
                        nc.tensor.matmul(
                            pc1[blk][:],
                            lhsT,
                            vtc[kt_i // 4][:, kt_i % 4, 0:512],
                            start=(kt_i == 0),
                            stop=last,
                        )
                        nc.tensor.matmul(
                            pc2[blk][:],
                            lhsT,
                            vtc[kt_i // 4][:, kt_i % 4, 512:769],
                            start=(kt_i == 0),
                            stop=last,
                        )
                        if last:
                            # normalize + store this q-block immediately; the
                            # two column halves use different engines (DVE /
                            # Act); out DMAs ride the HWDGE (sync) queue.
                            rinv = small_p.tile([128, 1], F32, tag="rinv")
                            nc.vector.reciprocal(rinv[:], pc2[blk][:, 256:257])
                            r = (2 * p + blk) * 128
                            osb1 = out_p.tile([128, 512], F32, tag="osb1")
                            nc.vector.tensor_mul(
                                osb1[:], pc1[blk][:], rinv[:].to_broadcast((128, 512))
                            )
                            nc.sync.dma_start(out_d[r : r + 128, 0:512], osb1[:])
                            osb2 = out_p.tile([128, 256], F32, tag="osb2")
                            nc.vector.tensor_mul(
                                osb2[:], pc2[blk][:, 0:256], rinv[:].to_broadcast((128, 256))
                            )
                            nc.gpsimd.dma_start(out_d[r : r + 128, 512:768], osb2[:])

            for p in range(4):
                pc1 = [
                    pc1a_p.tile([128, 512], F32, name="pc1a"),
                    pc1b_p.tile([128, 512], F32, name="pc1b"),
                ]
                pc2 = [
                    pc2a_p.tile([128, 257], F32, name="pc2a"),
                    pc2b_p.tile([128, 257], F32, name="pc2b"),
                ]
                nkt = [4 * p + 2, 4 * p + 4]  # ctx key-tile count per q-block
                S = 2 * p + 2  # steps (2 key-tiles each)
                ats = {}
                for it in range(S + LOOK):
                    if it < S:
                        ats[it] = emit_scores(p, it)
                    sc = it - LOOK
                    if sc >= 0:
                        emit_ctx(p, sc, ats.pop(sc), pc1, pc2, nkt)

    nc.compile()
    return nc


def _make_strip(h):
    """[128, 512] 0/1 mask; block i (128 wide) is added (via -1e30) to the
    masked q-block at diagonal offset i = kt - 4p. [key-row, query-col]."""
    tri = (np.arange(128)[:, None] > np.arange(128)[None, :]).astype(np.float32)
    ones = np.ones((128, 128), np.float32)
    zeros = np.zeros((128, 128), np.float32)
    blocks = [tri, ones, tri, ones] if h == 0 else [zeros, tri, zeros, tri]
    return np.concatenate(blocks, axis=1)


def _hi_lo(a):
    hi = a.astype(NP_FP8)
    lo = (a - hi.astype(np.float32)).astype(NP_FP8)
    return hi, lo


def kernel(x, Wq, Wk, Wv):
    if "nc" not in _CACHE:
        _CACHE["nc"] = _build()
    nc = _CACHE["nc"]

    x = np.ascontiguousarray(x, dtype=np.float32)
    wq8, wql8 = _hi_lo(WS * np.asarray(Wq, dtype=np.float32).T)
    wk8, wkl8 = _hi_lo(WS * np.asarray(Wk, dtype=np.float32).T)
    wvh8, wvl8 = _hi_lo(WS * np.asarray(Wv, dtype=np.float32).T)
    negd = (NEG * np.eye(128, dtype=np.float32)).astype(NP_BF16)

    in_maps = []
    for c in range(8):
        b, h = c // 2, c % 2
        xbt = np.ascontiguousarray(x[b].T)  # [768, 2048]
        xt8, xlo8 = _hi_lo(xbt)
        # own query columns: pairs p -> global tiles (4p+h, 4p+2+h)
        cols = []
        for p in range(4):
            for g in (4 * p + h, 4 * p + 2 + h):
                cols.append(xbt[:, g * 128 : (g + 1) * 128])
        xqt8 = np.ascontiguousarray(np.concatenate(cols, axis=1)).astype(NP_FP8)
        in_maps.append(
            {
                "xt": xt8,
                "xlo": xlo8,
                "xqt": xqt8,
                "wq": wq8,
                "wql": wql8,
                "wk": wk8,
                "wkl": wkl8,
                "wvh": wvh8,
                "wvl": wvl8,
                "strip": _make_strip(h).astype(NP_BF16),
                "negd": negd,
            }
        )

    res = run_bass_kernel_spmd(
        nc,
        in_maps,
        list(range(8)),
        trace=bool(int(os.environ.get("KERNEL_TRACE", "0"))),
    )
    _CACHE["last_results"] = res

    out = np.empty((BATCH, SEQ, D), np.float32)
    for c in range(8):
        b, h = c // 2, c % 2
        o = res.results[c]["out"]
        for p in range(4):
            for blk, g in enumerate((4 * p + h, 4 * p + 2 + h)):
                out[b, g * 128 : (g + 1) * 128] = o[
                    (2 * p + blk) * 128 : (2 * p + blk + 1) * 128
                ]
    return out
